# revision 1
# baseline (speedup 1.0000x reference)
"""MHA Trainium2 kernel: head-sharded tensor parallel across 8 NeuronCores.

Problem: B=2, S=2048, D=2560, H=32 heads, HD=80, partial rotary RD=32,
causal attention (-10000 mask), fp32.

Per-core plan (4 heads each):
  host: xT_aug tiles (x transposed + ones row for bias), per-core weight
        slices transposed, rope tables, causal mask tiles.
  A1: qkv projection for q,k in "natural" [s, j] layout via PE matmuls
      (K = embed dim, contraction tiles of 128), PE-transpose per head to
      qT/kT [80, s], then rope on DVE.
  A2: same for v, kept natural [s, 80] with an appended ones column.
  C:  per (head, q-block): scoresT [k,q] = kT.T-slice @ qT-slice on PE,
      exp on ACT, causal mask multiply on diagonal tiles (DVE),
      PV accumulate [81, qb] (row 80 = softmax denominator via ones col),
      reciprocal + PE-broadcast + DVE multiply -> ctxT [80, s].
  D:  out-proj yT [d, s] partials (head contraction), DMA out.
  host: sum partials over cores, transpose, add out bias.
"""
import sys
import os

sys.path.insert(0, "/opt/trn_rl_repo")

import numpy as np
from contextlib import ExitStack

import concourse.bacc as bacc
import concourse.tile as tile
import concourse.mybir as mybir
from concourse.bass_utils import run_bass_kernel_spmd
from concourse.masks import make_identity

F32 = mybir.dt.float32
F32R = mybir.dt.float32r

B, S, D = 2, 2048, 2560
H, HD = 32, 80
RD = 32
ROPE_BASE = 10000.0
N_CORES = 8


def rne11(x):
    """Round-to-nearest-even to 11 mantissa bits (matches HW f32r rounding)."""
    xi = np.ascontiguousarray(x, dtype=np.float32).view(np.uint32).astype(np.uint64)
    shift = np.uint64(12)
    bias = np.uint64((1 << 11) - 1)
    lsb = (xi >> shift) & np.uint64(1)
    xi = (xi + bias + lsb) >> shift << shift
    return xi.astype(np.uint32).view(np.float32)


def make_cfg(s=S, d=D, nh=H // N_CORES, hd=HD, rd=RD, b=B, qb=512, dt_mm="f32"):
    cfg = dict(s=s, d=d, nh=nh, hd=hd, rd=rd, b=b, qb=qb, dt_mm=dt_mm)
    cfg["n_st"] = s // 128
    cfg["n_kt"] = (d + 1 + 127) // 128
    cfg["d_aug"] = cfg["n_kt"] * 128
    cfg["nqb"] = s // qb
    cfg["ndiag"] = qb // 128
    cfg["n_dt"] = d // 128
    cfg["jq"] = nh * hd
    return cfg


def build_program(cfg):
    s, d, nh, hd, rd = cfg["s"], cfg["d"], cfg["nh"], cfg["hd"], cfg["rd"]
    qb, n_st, n_kt = cfg["qb"], cfg["n_st"], cfg["n_kt"]
    nqb, ndiag, n_dt, jq = cfg["nqb"], cfg["ndiag"], cfg["n_dt"], cfg["jq"]
    nb = cfg["b"]
    DT = F32 if cfg["dt_mm"] == "f32" else F32R
    rh = rd // 2

    nc = bacc.Bacc(None, debug=False)

    xs_d = [
        nc.declare_dram_parameter(f"xs_b{b}", [n_st, 128, n_kt * 128], DT,
                                  isOutput=False)
        for b in range(nb)
    ]
    wqk_d = nc.declare_dram_parameter("wqk", [128, n_kt, 2 * jq], DT,
                                      isOutput=False)
    wv_d = nc.declare_dram_parameter("wv", [128, n_kt, jq], DT, isOutput=False)
    outw_d = nc.declare_dram_parameter("outw", [nh, hd, d], DT, isOutput=False)
    cos_d = nc.declare_dram_parameter("cosN", [128, n_st * rh], F32,
                                      isOutput=False)
    sin_d = nc.declare_dram_parameter("sinN", [128, n_st * rh], F32,
                                      isOutput=False)
    onecol = ((hd + 31) // 32) * 32  # 32-aligned ones column in v_aug
    mask_d = nc.declare_dram_parameter("masks", [128, ndiag * qb], DT,
                                       isOutput=False)
    y_d = [
        nc.declare_dram_parameter(f"y_b{b}", [d, s], F32, isOutput=True)
        for b in range(nb)
    ]
    if cfg.get("dump"):
        dq_d = nc.declare_dram_parameter("dump_q", [hd, s], F32, isOutput=True)
        dk_d = nc.declare_dram_parameter("dump_k", [hd, s], F32, isOutput=True)
        dv_d = nc.declare_dram_parameter("dump_v", [n_st, 128, onecol + 1], F32,
                                         isOutput=True)
        dc_d = nc.declare_dram_parameter("dump_c", [hd, s], F32, isOutput=True)
        dcos_d = nc.declare_dram_parameter("dump_cos", [128, n_st * rh], F32,
                                           isOutput=True)

    with tile.TileContext(nc) as tc, ExitStack() as top:
        glob = top.enter_context(tc.tile_pool(name="glob", bufs=1))
        identf = glob.tile([128, 128], F32)
        make_identity(nc, identf)
        if DT is F32:
            ident = identf
        else:
            ident = glob.tile([128, 128], DT)
            nc.vector.tensor_copy(ident, identf)
        vpad = glob.tile([128, onecol + 1 - hd], F32)
        nc.vector.memset(vpad, 0.0)
        nc.vector.memset(vpad[:, onecol - hd:onecol + 1 - hd], 1.0)
        ones1 = glob.tile([1, hd], F32)
        nc.vector.memset(ones1, 1.0)
        cosN = glob.tile([128, n_st * rh], F32)
        nc.sync.dma_start(out=cosN, in_=cos_d[:, :])
        sinN = glob.tile([128, n_st * rh], F32)
        nc.sync.dma_start(out=sinN, in_=sin_d[:, :])
        masks = glob.tile([128, ndiag * qb], DT)
        nc.sync.dma_start(out=masks, in_=mask_d[:, :])

        for b in range(nb):
            with ExitStack() as bstk:
                qt_pool = bstk.enter_context(
                    tc.tile_pool(name=f"qt{b}", bufs=1))
                qT = [qt_pool.tile([hd, s], DT, tag=f"q{h}", name=f"qT{b}_{h}") for h in range(nh)]
                kT = [qt_pool.tile([hd, s], DT, tag=f"k{h}", name=f"kT{b}_{h}") for h in range(nh)]

                # ---- phase A1: q,k projection + transpose ----
                with ExitStack() as a1:
                    wp = a1.enter_context(tc.tile_pool(name=f"w1_{b}", bufs=1))
                    wqk = wp.tile([128, n_kt, 2 * jq], DT)
                    nc.sync.dma_start(out=wqk, in_=wqk_d[:, :, :])
                    xsp = a1.enter_context(tc.tile_pool(name=f"xs1_{b}", bufs=2))
                    qkn = a1.enter_context(tc.tile_pool(name=f"qkn{b}", bufs=2))
                    psA = a1.enter_context(
                        tc.tile_pool(name=f"psA{b}", bufs=2, space="PSUM"))
                    psT = a1.enter_context(
                        tc.tile_pool(name=f"psT{b}", bufs=2, space="PSUM"))
                    rtp = a1.enter_context(tc.tile_pool(name=f"rt{b}", bufs=2))
                    for st in range(n_st):
                        xs = xsp.tile([128, n_kt * 128], DT, tag="xs")
                        nc.sync.dma_start(out=xs, in_=xs_d[b][st])
                        xs3 = xs.rearrange("p (t c) -> p t c", t=n_kt)
                        stage = qkn.tile([128, 2 * jq], DT, tag="qkn")
                        for blk in range(2):
                            ps = psA.tile([128, jq], F32, tag="ps")
                            for kt in range(n_kt):
                                nc.tensor.matmul(
                                    ps,
                                    xs3[:, kt, :],
                                    wqk[:, kt, blk * jq:(blk + 1) * jq],
                                    start=(kt == 0),
                                    stop=(kt == n_kt - 1),
                                )
                            nc.scalar.copy(stage[:, blk * jq:(blk + 1) * jq], ps)
                        cN = cosN[:, st * rh:(st + 1) * rh]
                        sN = sinN[:, st * rh:(st + 1) * rh]
                        for h in range(nh):
                            for qk in range(2):
                                base = qk * jq + h * hd
                                t1 = stage[:, base:base + rh]
                                t2 = stage[:, base + rh:base + rd]
                                ta = rtp.tile([128, rh], F32, tag="ta")
                                nc.vector.tensor_mul(ta, t1, cN)
                                tb = rtp.tile([128, rh], F32, tag="tb")
                                nc.vector.tensor_mul(tb, t2, sN)
                                tg = rtp.tile([128, rh], F32, tag="tg")
                                nc.vector.tensor_mul(tg, t1, sN)
                                td = rtp.tile([128, rh], F32, tag="td")
                                nc.vector.tensor_mul(td, t2, cN)
                                nc.vector.tensor_sub(t1, ta, tb)
                                nc.vector.tensor_add(t2, tg, td)
                        for h in range(nh):
                            for qk, dstT in ((0, qT), (1, kT)):
                                pt = psT.tile([hd, 128], DT, tag="pt")
                                nc.tensor.transpose(
                                    pt,
                                    stage[:, qk * jq + h * hd:
                                          qk * jq + (h + 1) * hd],
                                    ident,
                                )
                                nc.vector.tensor_copy(
                                    dstT[h][:, st * 128:(st + 1) * 128], pt)

                # ---- phase A2: v projection (natural + ones col) ----
                vp = bstk.enter_context(tc.tile_pool(name=f"v{b}", bufs=1))
                vA = [
                    [vp.tile([128, onecol + 1], DT, tag=f"v{h}_{st}",
                             name=f"vA{b}_{h}_{st}")
                     for st in range(n_st)]
                    for h in range(nh)
                ]
                with ExitStack() as a2:
                    wp2 = a2.enter_context(tc.tile_pool(name=f"w2_{b}", bufs=1))
                    wv = wp2.tile([128, n_kt, jq], DT)
                    nc.sync.dma_start(out=wv, in_=wv_d[:, :, :])
                    xsp2 = a2.enter_context(tc.tile_pool(name=f"xs2_{b}", bufs=2))
                    psA2 = a2.enter_context(
                        tc.tile_pool(name=f"psA2{b}", bufs=2, space="PSUM"))
                    for st in range(n_st):
                        xs = xsp2.tile([128, n_kt * 128], DT, tag="xs")
                        nc.sync.dma_start(out=xs, in_=xs_d[b][st])
                        xs3 = xs.rearrange("p (t c) -> p t c", t=n_kt)
                        ps = psA2.tile([128, jq], F32, tag="ps")
                        for kt in range(n_kt):
                            nc.tensor.matmul(
                                ps,
                                xs3[:, kt, :],
                                wv[:, kt, :],
                                start=(kt == 0),
                                stop=(kt == n_kt - 1),
                            )
                        for h in range(nh):
                            nc.scalar.copy(
                                vA[h][st][:, 0:hd],
                                ps[:, h * hd:(h + 1) * hd])
                            nc.vector.tensor_copy(
                                vA[h][st][:, hd:onecol + 1], vpad)

                if cfg.get("dump") and b == 0:
                    nc.sync.dma_start(out=dcos_d[:, :], in_=cosN)
                    nc.sync.dma_start(out=dq_d[:, :], in_=qT[0])
                    nc.sync.dma_start(out=dk_d[:, :], in_=kT[0])
                    for st in range(n_st):
                        nc.sync.dma_start(out=dv_d[st], in_=vA[0][st])

                # ---- phase C: attention ----
                ctx_pool = bstk.enter_context(tc.tile_pool(name=f"ctx{b}", bufs=1))
                ctxT = [ctx_pool.tile([hd, s], DT, tag=f"c{h}", name=f"ctxT{b}_{h}") for h in range(nh)]
                with ExitStack() as cstk:
                    pp = cstk.enter_context(tc.tile_pool(name=f"pT{b}", bufs=3))
                    rp2 = cstk.enter_context(tc.tile_pool(name=f"rr{b}", bufs=2))
                    psS = cstk.enter_context(
                        tc.tile_pool(name=f"psS{b}", bufs=2, space="PSUM"))
                    psC = cstk.enter_context(
                        tc.tile_pool(name=f"psC{b}", bufs=2, space="PSUM"))
                    psB = cstk.enter_context(
                        tc.tile_pool(name=f"psB{b}", bufs=2, space="PSUM"))
                    for h in range(nh):
                        for q0 in range(nqb):
                            nkt_q = (q0 + 1) * qb // 128
                            pctx = psC.tile([onecol + 1, qb], F32, tag="pc")
                            for kt in range(nkt_q):
                                pss = psS.tile([128, qb], F32, tag="ps")
                                nc.tensor.matmul(
                                    pss,
                                    kT[h][:, kt * 128:(kt + 1) * 128],
                                    qT[h][:, q0 * qb:(q0 + 1) * qb],
                                    start=True, stop=True,
                                )
                                pT = pp.tile([128, qb], DT, tag="p")
                                nc.scalar.activation(
                                    pT, pss, mybir.ActivationFunctionType.Exp)
                                od = kt * 128 - q0 * qb
                                if od >= 0:
                                    oi = od // 128
                                    nc.vector.tensor_mul(
                                        pT, pT,
                                        masks[:, oi * qb:(oi + 1) * qb])
                                nc.tensor.matmul(
                                    pctx, vA[h][kt], pT,
                                    start=(kt == 0), stop=(kt == nkt_q - 1),
                                )
                            rden = rp2.tile([1, qb], F32, tag="rd")
                            nc.vector.reciprocal(rden, pctx[onecol:onecol + 1, :])
                            pbc = psB.tile([hd, qb], F32, tag="bc")
                            nc.tensor.matmul(pbc, ones1, rden,
                                             start=True, stop=True)
                            rb = rp2.tile([hd, qb], F32, tag="rb")
                            nc.scalar.copy(rb, pbc)
                            nc.vector.tensor_mul(
                                ctxT[h][:, q0 * qb:(q0 + 1) * qb],
                                pctx[0:hd, :], rb)

                if cfg.get("dump") and b == 0:
                    nc.sync.dma_start(out=dc_d[:, :], in_=ctxT[0])

                # ---- phase D: out projection ----
                with ExitStack() as dstk:
                    op = dstk.enter_context(tc.tile_pool(name=f"ow{b}", bufs=1))
                    ow = [op.tile([hd, d], DT, tag=f"o{h}", name=f"ow{b}_{h}") for h in range(nh)]
                    for h in range(nh):
                        nc.sync.dma_start(out=ow[h], in_=outw_d[h])
                    stp = dstk.enter_context(tc.tile_pool(name=f"st{b}", bufs=2))
                    psD = dstk.enter_context(
                        tc.tile_pool(name=f"psD{b}", bufs=2, space="PSUM"))
                    for dt_i in range(n_dt):
                        stage = stp.tile([128, s], F32, tag="y")
                        for sb in range(nqb):
                            psy = psD.tile([128, qb], F32, tag="ps")
                            for h in range(nh):
                                nc.tensor.matmul(
                                    psy,
                                    ow[h][:, dt_i * 128:(dt_i + 1) * 128],
                                    ctxT[h][:, sb * qb:(sb + 1) * qb],
                                    start=(h == 0), stop=(h == nh - 1),
                                )
                            nc.scalar.copy(stage[:, sb * qb:(sb + 1) * qb], psy)
                        nc.sync.dma_start(
                            out=y_d[b][dt_i * 128:(dt_i + 1) * 128, :],
                            in_=stage)

    nc.finalize()
    return nc


def prep_core_inputs(cfg, x, wqkv_w, wqkv_b, out_w, core):
    s, d, nh, hd, rd = cfg["s"], cfg["d"], cfg["nh"], cfg["hd"], cfg["rd"]
    qb, n_st, n_kt, d_aug = cfg["qb"], cfg["n_st"], cfg["n_kt"], cfg["d_aug"]
    ndiag, jq = cfg["ndiag"], cfg["jq"]
    nb = cfg["b"]
    rh = rd // 2
    rnd = rne11 if cfg["dt_mm"] == "f32r" else (lambda a: np.asarray(a, np.float32))

    heads = range(core * nh, (core + 1) * nh)
    rows = np.concatenate([np.arange(h * hd, (h + 1) * hd) for h in heads])
    scale = np.float32(1.0 / np.sqrt(hd))

    wq = wqkv_w[rows, :]
    bq = wqkv_b[rows]
    wk = wqkv_w[d + rows, :] * scale
    bk = wqkv_b[d + rows] * scale
    wv = wqkv_w[2 * d + rows, :]
    bv = wqkv_b[2 * d + rows]

    def wt_tiles(w, bias):
        # [d_aug, J] with row d = bias, rows > d zero -> [128, n_kt, J]
        j = w.shape[0]
        wa = np.zeros((d_aug, j), np.float32)
        wa[:d] = w.T
        wa[d] = bias
        return np.ascontiguousarray(
            wa.reshape(n_kt, 128, j).transpose(1, 0, 2))

    wqk_arr = rnd(np.concatenate([wt_tiles(wq, bq), wt_tiles(wk, bk)], axis=2))
    wv_arr = rnd(wt_tiles(wv, bv))

    outw = np.ascontiguousarray(out_w[:, rows].T.reshape(nh, hd, d))
    outw_arr = rnd(outw)

    inv_freq = 1.0 / (ROPE_BASE ** (np.arange(0, rd, 2, dtype=np.float32) / rd))
    t = np.arange(s, dtype=np.float32)
    freqs = np.outer(t, inv_freq)
    n_st_ = cfg["n_st"]
    cos_arr = np.ascontiguousarray(
        np.cos(freqs).astype(np.float32).reshape(n_st_, 128, rh)
        .transpose(1, 0, 2).reshape(128, n_st_ * rh))
    sin_arr = np.ascontiguousarray(
        np.sin(freqs).astype(np.float32).reshape(n_st_, 128, rh)
        .transpose(1, 0, 2).reshape(128, n_st_ * rh))

    km = np.arange(128)[:, None]
    qm = np.arange(qb)[None, :]
    mask_arr = np.concatenate(
        [(qm >= i * 128 + km).astype(np.float32) for i in range(ndiag)],
        axis=1)
    mask_arr = np.ascontiguousarray(mask_arr)

    in_map = {
        "wqk": wqk_arr, "wv": wv_arr, "outw": outw_arr,
        "cosN": cos_arr, "sinN": sin_arr, "masks": mask_arr,
    }
    for bi in range(nb):
        xa = np.zeros((d_aug, s), np.float32)
        xa[:d] = x[bi].T
        xa[d] = 1.0
        in_map[f"xs_b{bi}"] = rnd(
            np.ascontiguousarray(
                xa.reshape(n_kt, 128, n_st, 128).transpose(2, 1, 0, 3)
            ).reshape(n_st, 128, n_kt * 128))
    return in_map


_CACHE = {}


def run_mha(cfg, x, wqkv_w, wqkv_b, out_w, out_b, trace=False):
    key = tuple(sorted(cfg.items()))
    if key not in _CACHE:
        _CACHE[key] = build_program(cfg)
    nc = _CACHE[key]
    in_maps = [
        prep_core_inputs(cfg, x, wqkv_w, wqkv_b, out_w, c)
        for c in range(N_CORES)
    ]
    res = run_bass_kernel_spmd(nc, in_maps, core_ids=list(range(N_CORES)),
                               trace=trace)
    nb, d, s = cfg["b"], cfg["d"], cfg["s"]
    y = np.zeros((nb, s, d), np.float32)
    for bi in range(nb):
        acc = np.zeros((d, s), np.float64)
        for c in range(N_CORES):
            acc += res.results[c][f"y_b{bi}"]
        y[bi] = acc.T.astype(np.float32) + out_b[None, :]
    return y, res


def kernel(x, wqkv_w, wqkv_b, out_w, out_b):
    cfg = make_cfg(dt_mm=os.environ.get("KMHA_DT", "f32"))
    y, _ = run_mha(cfg, np.asarray(x, np.float32), np.asarray(wqkv_w, np.float32),
                   np.asarray(wqkv_b, np.float32), np.asarray(out_w, np.float32),
                   np.asarray(out_b, np.float32))
    return y



# revision 3
# speedup vs baseline: 1.1674x; 1.1674x over previous
"""MHA Trainium2 kernel v2: (batch x head-group) sharded across 8 NeuronCores.

Problem: B=2, S=2048, D=2560, H=32 heads, HD=80, partial rotary RD=32,
causal attention, fp32 reference; kernel computes in bf16 with f32 PSUM.

Core c handles batch c//4, heads (c%4)*8 .. +8.

Per-core phases:
  A1: q,k projection (x-tile stationary, weight moving) -> stage [tok, 1280],
      rope on DVE (3D strided APs batching all 8 heads), PE-transpose per head
      to qT/kT [80, s].
  A2: v projection -> stage_v [tok, 8*(80+1)] resident; per-head col 80 is a
      bias-only "ones" channel so PV yields the softmax denominator.
  C:  per (head, q-block): scoresT [k,q] on PE, exp on ACT (f32 PSUM -> bf16),
      causal mask multiply on diagonal tiles (DVE), PV accumulate [81, qb],
      reciprocal of row 80 + PE-broadcast + DVE multiply -> ctxT [80, s].
      ctxT repacked into [128, s] tiles via SBUF->SBUF DMA per head.
  D:  out-proj yT [d, s] partial (K=128 packed ctx), DMA out bf16.
  host: sum 4 partials per batch in f32, transpose, add out bias.
"""
import sys
import os

sys.path.insert(0, "/opt/trn_rl_repo")

import numpy as np
from contextlib import ExitStack

import concourse.bacc as bacc
import concourse.tile as tile
import concourse.mybir as mybir
from concourse.bass_utils import run_bass_kernel_spmd
from concourse.masks import make_identity

F32 = mybir.dt.float32
F32R = mybir.dt.float32r
BF16 = mybir.dt.bfloat16

B, S, D = 2, 2048, 2560
H, HD = 32, 80
RD = 32
ROPE_BASE = 10000.0
N_CORES = 8
NBG = 4  # cores per batch


def make_cfg(s=S, d=D, nh=H // NBG, hd=HD, rd=RD, qb=512, dt="bf16"):
    cfg = dict(s=s, d=d, nh=nh, hd=hd, rd=rd, qb=qb, dt=dt)
    cfg["n_st"] = s // 128
    cfg["n_kt"] = d // 128
    cfg["d_aug"] = cfg["n_kt"] * 128
    cfg["nqb"] = s // qb
    cfg["ndiag"] = qb // 128
    cfg["n_dt"] = d // 128
    cfg["jqk"] = 2 * nh * hd          # 1280
    cfg["vw"] = ((hd + 31) // 32) * 32 + 1  # 97: ones channel at 32-aligned col
    cfg["jv"] = nh * cfg["vw"]              # 776
    cfg["n_ct"] = (nh * hd) // 128    # 5 packed ctx tiles
    return cfg


def _dt(cfg):
    return {"bf16": BF16, "f32": F32, "f32r": F32R}[cfg["dt"]]


def _groups(total, step=512):
    out = []
    a = 0
    while a < total:
        out.append((a, min(a + step, total)))
        a += step
    return out


def build_program(cfg):
    s, d, nh, hd, rd = cfg["s"], cfg["d"], cfg["nh"], cfg["hd"], cfg["rd"]
    qb, n_st, n_kt = cfg["qb"], cfg["n_st"], cfg["n_kt"]
    nqb, ndiag, n_dt = cfg["nqb"], cfg["ndiag"], cfg["n_dt"]
    jqk, vw, jv, n_ct = cfg["jqk"], cfg["vw"], cfg["jv"], cfg["n_ct"]
    DT = _dt(cfg)
    rh = rd // 2

    nc = bacc.Bacc(None, debug=False)

    xs_d = nc.declare_dram_parameter("xs", [n_st, 128, n_kt * 128], DT,
                                     isOutput=False)
    wqk_d = nc.declare_dram_parameter("wqk", [n_kt, 128, jqk], DT,
                                      isOutput=False)
    wv_d = nc.declare_dram_parameter("wv", [n_kt, 128, jv], DT, isOutput=False)
    outw_d = nc.declare_dram_parameter("outw", [n_ct, 128, d], DT,
                                       isOutput=False)
    cos_d = nc.declare_dram_parameter("cosR", [128, n_st * nh * rh], DT,
                                      isOutput=False)
    sin_d = nc.declare_dram_parameter("sinR", [128, n_st * nh * rh], DT,
                                      isOutput=False)
    mask_d = nc.declare_dram_parameter("masks", [128, ndiag * qb], DT,
                                       isOutput=False)
    bqk_d = nc.declare_dram_parameter("bqk", [128, jqk], DT, isOutput=False)
    bv_d = nc.declare_dram_parameter("bv", [128, jv], DT, isOutput=False)
    y_d = nc.declare_dram_parameter("y", [d, s], DT, isOutput=True)

    with tile.TileContext(nc) as tc, ExitStack() as top:
        top.enter_context(
            nc.allow_low_precision(reason="intentional bf16 storage"))
        glob = top.enter_context(tc.tile_pool(name="glob", bufs=1))
        identf = glob.tile([128, 128], F32)
        make_identity(nc, identf)
        if DT is F32:
            ident = identf
        else:
            ident = glob.tile([128, 128], DT)
            nc.vector.tensor_copy(ident, identf)
        ones1f = glob.tile([1, hd], F32)
        nc.vector.memset(ones1f, 1.0)
        ones1 = glob.tile([1, hd], F32R)
        nc.vector.tensor_copy(ones1, ones1f)
        cosR = glob.tile([128, n_st * nh * rh], DT)
        sinR = glob.tile([128, n_st * nh * rh], DT)
        masks = glob.tile([128, ndiag * qb], DT)
        bqk = glob.tile([128, jqk], DT)
        bv = glob.tile([128, jv], DT)

        qt_pool = top.enter_context(tc.tile_pool(name="qt", bufs=1))
        qT = [qt_pool.tile([hd, s], DT, tag=f"q{h}", name=f"qT{h}")
              for h in range(nh)]
        kT = [qt_pool.tile([hd, s], DT, tag=f"k{h}", name=f"kT{h}")
              for h in range(nh)]
        vp = top.enter_context(tc.tile_pool(name="vp", bufs=1))
        vA = [vp.tile([128, jv], DT, tag=f"v{st}", name=f"vA{st}")
              for st in range(n_st)]

        # ---- phase A1: q,k projection + rope + transpose ----
        qk_groups = _groups(jqk)
        xsp = top.enter_context(tc.tile_pool(name="xsp", bufs=2))
        xs0 = xsp.tile([128, n_kt * 128], DT, tag="xs", name="xs_a1_0")
        nc.sync.dma_start(out=xs0, in_=xs_d[0])
        with ExitStack() as a1:
            wp = a1.enter_context(tc.tile_pool(name="w1", bufs=1))
            wqk = [wp.tile([128, jqk], DT, tag=f"w1_{kt}", name=f"wqk{kt}")
                   for kt in range(n_kt)]
            for kt in range(4):
                nc.sync.dma_start(out=wqk[kt], in_=wqk_d[kt])
            nc.sync.dma_start(out=cosR, in_=cos_d[:, :])
            nc.sync.dma_start(out=sinR, in_=sin_d[:, :])
            nc.sync.dma_start(out=masks, in_=mask_d[:, :])
            nc.sync.dma_start(out=bqk, in_=bqk_d[:, :])
            nc.sync.dma_start(out=bv, in_=bv_d[:, :])
            for kt in range(4, n_kt):
                nc.sync.dma_start(out=wqk[kt], in_=wqk_d[kt])
            stp = a1.enter_context(tc.tile_pool(name="stg1", bufs=2))
            psA = a1.enter_context(
                tc.tile_pool(name="psA", bufs=2, space="PSUM"))
            psT = a1.enter_context(
                tc.tile_pool(name="psT", bufs=2, space="PSUM"))
            rtp = a1.enter_context(tc.tile_pool(name="rt", bufs=2))
            for st in range(n_st):
                if st == 0:
                    xs = xs0
                else:
                    xs = xsp.tile([128, n_kt * 128], DT, tag="xs",
                                  name=f"xs_a1_{st}")
                    nc.sync.dma_start(out=xs, in_=xs_d[st])
                xs3 = xs.rearrange("p (t c) -> p t c", t=n_kt)
                ps = [psA.tile([128, g1 - g0], F32, tag=f"ps{gi}", name=f"psA{st}_{gi}")
                      for gi, (g0, g1) in enumerate(qk_groups)]
                for kt in range(n_kt):
                    for gi, (g0, g1) in enumerate(qk_groups):
                        nc.tensor.matmul(
                            ps[gi], xs3[:, kt, :], wqk[kt][:, g0:g1],
                            start=(kt == 0), stop=(kt == n_kt - 1))
                stage = stp.tile([128, jqk], DT, tag="stage")
                for gi, (g0, g1) in enumerate(qk_groups):
                    nc.vector.scalar_tensor_tensor(
                        out=stage[:, g0:g1], in0=ps[gi], scalar=1.0,
                        in1=bqk[:, g0:g1], op0=mybir.AluOpType.mult,
                        op1=mybir.AluOpType.add)
                # rope: all nh heads per op via strided 3D views
                cN = cosR[:, st * nh * rh:(st + 1) * nh * rh] \
                    .rearrange("p (h c) -> p h c", h=nh)
                sN = sinR[:, st * nh * rh:(st + 1) * nh * rh] \
                    .rearrange("p (h c) -> p h c", h=nh)
                for qk in range(2):
                    blk = stage[:, qk * nh * hd:(qk + 1) * nh * hd] \
                        .rearrange("p (h c) -> p h c", h=nh)
                    t1 = blk[:, :, 0:rh]
                    t2 = blk[:, :, rh:rd]
                    ta = rtp.tile([128, nh, rh], F32, tag="ta")
                    tb = rtp.tile([128, nh, rh], F32, tag="tb")
                    tg = rtp.tile([128, nh, rh], F32, tag="tg")
                    td = rtp.tile([128, nh, rh], F32, tag="td")
                    nc.vector.tensor_mul(ta, t1, cN)
                    nc.vector.tensor_mul(tb, t2, sN)
                    nc.vector.tensor_mul(tg, t1, sN)
                    nc.vector.tensor_mul(td, t2, cN)
                    nc.vector.tensor_sub(t1, ta, tb)
                    nc.vector.tensor_add(t2, tg, td)
                for h in range(nh):
                    for qk, dstT in ((0, qT), (1, kT)):
                        pt = psT.tile([hd, 128], DT, tag="pt")
                        nc.tensor.transpose(
                            pt, stage[:, qk * nh * hd + h * hd:
                                      qk * nh * hd + (h + 1) * hd], ident)
                        nc.vector.tensor_copy(
                            dstT[h][:, st * 128:(st + 1) * 128], pt)

        # ---- phase A2: v projection (+ ones channel per head) ----
        v_groups = _groups(jv)
        with ExitStack() as a2:
            wp2 = a2.enter_context(tc.tile_pool(name="w2", bufs=1))
            wv = [wp2.tile([128, jv], DT, tag=f"w2_{kt}", name=f"wv{kt}")
                  for kt in range(n_kt)]
            for kt in range(n_kt):
                nc.sync.dma_start(out=wv[kt], in_=wv_d[kt])
            psA2 = a2.enter_context(
                tc.tile_pool(name="psA2", bufs=2, space="PSUM"))
            for st in range(n_st):
                xs = xsp.tile([128, n_kt * 128], DT, tag="xs",
                              name=f"xs_a2_{st}")
                nc.sync.dma_start(out=xs, in_=xs_d[st])
                xs3 = xs.rearrange("p (t c) -> p t c", t=n_kt)
                ps = [psA2.tile([128, g1 - g0], F32, tag=f"ps{gi}", name=f"psA2{st}_{gi}")
                      for gi, (g0, g1) in enumerate(v_groups)]
                for kt in range(n_kt):
                    for gi, (g0, g1) in enumerate(v_groups):
                        nc.tensor.matmul(
                            ps[gi], xs3[:, kt, :], wv[kt][:, g0:g1],
                            start=(kt == 0), stop=(kt == n_kt - 1))
                for gi, (g0, g1) in enumerate(v_groups):
                    nc.vector.scalar_tensor_tensor(
                        out=vA[st][:, g0:g1], in0=ps[gi], scalar=1.0,
                        in1=bv[:, g0:g1], op0=mybir.AluOpType.mult,
                        op1=mybir.AluOpType.add)

        # ---- phase C: attention ----
        ctx_pool = top.enter_context(tc.tile_pool(name="ctx", bufs=1))
        ctxT = [ctx_pool.tile([hd, s], DT, tag=f"c{h}", name=f"ctxT{h}")
                for h in range(nh)]
        ctxP = [ctx_pool.tile([128, s], DT, tag=f"cp{t}", name=f"ctxP{t}")
                for t in range(n_ct)]

        def repack(h, c0, c1):
            g0 = h * hd
            r = g0
            while r < g0 + hd:
                ct = r // 128
                r1 = min((ct + 1) * 128, g0 + hd)
                nc.sync.dma_start(
                    out=ctxP[ct][r - ct * 128:r1 - ct * 128, c0:c1],
                    in_=ctxT[h][r - g0:r1 - g0, c0:c1])
                r = r1

        op = top.enter_context(tc.tile_pool(name="ow", bufs=1))
        ow = [op.tile([128, d], DT, tag=f"o{t}", name=f"ow{t}")
              for t in range(n_ct)]
        for t in range(n_ct):
            nc.sync.dma_start(out=ow[t], in_=outw_d[t])

        with ExitStack() as cstk:
            pp = cstk.enter_context(tc.tile_pool(name="pT", bufs=3))
            rp2 = cstk.enter_context(tc.tile_pool(name="rr", bufs=1))
            psS = cstk.enter_context(
                tc.tile_pool(name="psS", bufs=2, space="PSUM"))
            psC = cstk.enter_context(
                tc.tile_pool(name="psC", bufs=1, space="PSUM"))
            psD = cstk.enter_context(
                tc.tile_pool(name="psD", bufs=2, space="PSUM"))
            stp2 = cstk.enter_context(tc.tile_pool(name="st2", bufs=2))

            def finalize(h, q0, pctx):
                # copy den + ctx out of PSUM first: releases the pctx bank in
                # ~0.7us so the next block's accumulation can start while the
                # recip/broadcast/normalize chain runs from SBUF.
                den = rp2.tile([1, qb], F32, tag="rd")
                nc.vector.tensor_copy(den, pctx[vw - 1:vw, :])
                ctx_s = rp2.tile([hd, qb], DT, tag="cs")
                nc.vector.tensor_copy(ctx_s, pctx[0:hd, :])
                rdenr = rp2.tile([1, qb], F32R, tag="rdr")
                nc.vector.reciprocal(rdenr, den)
                pbc = psD.tile([hd, qb], F32, tag="ps")
                nc.tensor.matmul(pbc, ones1, rdenr, start=True, stop=True)
                rb = rp2.tile([hd, qb], DT, tag="rb")
                nc.vector.tensor_copy(rb, pbc)
                nc.vector.tensor_mul(
                    ctxT[h][:, q0 * qb:(q0 + 1) * qb], ctx_s, rb)
                repack(h, q0 * qb, (q0 + 1) * qb)

            # per head: two sweeps of q0-pairs; kt-outer within a sweep.
            # pctx[parity] lives in 1 PSUM bank each; pss2 [128, 2*qb]
            # spans 2 banks and holds both q0s of the sweep per kt.
            for sw in range(nqb // 2):
                q0s = (2 * sw, 2 * sw + 1)
                for h in range(nh):
                    pctxs = {
                        q0: psC.tile([vw, qb], F32, tag=f"pc{q0 % 2}",
                                     name=f"pctx{h}_{q0}")
                        for q0 in q0s
                    }
                    nkt_max = (q0s[1] + 1) * qb // 128
                    for kt in range(nkt_max):
                        act = [q0 for q0 in q0s
                               if kt < (q0 + 1) * qb // 128]
                        pss = {}
                        for ci, q0 in enumerate(act):
                            pss[q0] = psS.tile([128, qb], F32,
                                               tag=f"ss{q0 % 2}",
                                               name=f"pss{h}_{kt}_{q0}")
                            nc.tensor.matmul(
                                pss[q0],
                                kT[h][:, kt * 128:(kt + 1) * 128],
                                qT[h][:, q0 * qb:(q0 + 1) * qb],
                                start=True, stop=True)
                        pTs = {}
                        for ci, q0 in enumerate(act):
                            pTs[q0] = pp.tile([128, qb], DT, tag="p",
                                              name=f"pT{h}_{kt}_{q0}")
                            nc.scalar.activation(
                                pTs[q0], pss[q0],
                                mybir.ActivationFunctionType.Exp)
                            od = kt * 128 - q0 * qb
                            if od >= 0:
                                oi = od // 128
                                nc.vector.tensor_mul(
                                    pTs[q0], pTs[q0],
                                    masks[:, oi * qb:(oi + 1) * qb])
                        for ci, q0 in enumerate(act):
                            nkt_q = (q0 + 1) * qb // 128
                            nc.tensor.matmul(
                                pctxs[q0], vA[kt][:, h * vw:(h + 1) * vw],
                                pTs[q0],
                                start=(kt == 0), stop=(kt == nkt_q - 1))
                            if kt == nkt_q - 1:
                                finalize(h, q0, pctxs[q0])
                # ---- out projection for this token half, overlapped ----
                for dt_i in range(n_dt):
                    for sb in q0s:
                        psy = psD.tile([128, qb], F32, tag="ps")
                        for ct in range(n_ct):
                            nc.tensor.matmul(
                                psy, ow[ct][:, dt_i * 128:(dt_i + 1) * 128],
                                ctxP[ct][:, sb * qb:(sb + 1) * qb],
                                start=(ct == 0), stop=(ct == n_ct - 1))
                        ystage = stp2.tile([128, qb], DT, tag="y")
                        nc.vector.tensor_copy(ystage, psy)
                        nc.sync.dma_start(
                            out=y_d[dt_i * 128:(dt_i + 1) * 128,
                                    sb * qb:(sb + 1) * qb],
                            in_=ystage)

    nc.finalize()
    return nc


def prep_core_inputs(cfg, x, wqkv_w, wqkv_b, out_w, core):
    s, d, nh, hd, rd = cfg["s"], cfg["d"], cfg["nh"], cfg["hd"], cfg["rd"]
    qb, n_st, n_kt, d_aug = cfg["qb"], cfg["n_st"], cfg["n_kt"], cfg["d_aug"]
    ndiag, jqk, vw, jv = cfg["ndiag"], cfg["jqk"], cfg["vw"], cfg["jv"]
    n_ct = cfg["n_ct"]
    rh = rd // 2
    npdt = mybir.dt.np(_dt(cfg))

    bi = core // NBG
    hg = core % NBG
    heads = range(hg * nh, (hg + 1) * nh)
    rows = np.concatenate([np.arange(h * hd, (h + 1) * hd) for h in heads])
    scale = np.float32(1.0 / np.sqrt(hd))

    wq = wqkv_w[rows, :]
    bq = wqkv_b[rows]
    wk = wqkv_w[d + rows, :] * scale
    bk = wqkv_b[d + rows] * scale
    wv = wqkv_w[2 * d + rows, :]
    bv = wqkv_b[2 * d + rows]

    def wt_tiles(w):
        return np.ascontiguousarray(w.T).reshape(n_kt, 128, w.shape[0])

    wqk_arr = np.concatenate([wt_tiles(wq), wt_tiles(wk)], axis=2)
    bqk_arr = np.broadcast_to(
        np.concatenate([bq, bk])[None, :], (128, jqk))

    # v: per-head block of vw cols; col vw-1 is the "ones" channel whose
    # weights are zero and bias is 1 (softmax denominator trick)
    wva = np.zeros((d, jv), np.float32)
    bva = np.zeros((jv,), np.float32)
    for h in range(nh):
        wva[:, h * vw:h * vw + hd] = wv[h * hd:(h + 1) * hd].T
        bva[h * vw:h * vw + hd] = bv[h * hd:(h + 1) * hd]
        bva[h * vw + vw - 1] = 1.0
    wv_arr = wva.reshape(n_kt, 128, jv)
    bv_arr = np.broadcast_to(bva[None, :], (128, jv))

    outw_arr = np.ascontiguousarray(
        out_w[:, rows].T.reshape(n_ct, 128, d))

    inv_freq = 1.0 / (ROPE_BASE ** (np.arange(0, rd, 2, dtype=np.float32) / rd))
    t = np.arange(s, dtype=np.float32)
    freqs = np.outer(t, inv_freq)  # [s, rh]
    # [128, n_st, nh, rh]: value depends on (token=st*128+p, freq i); repl. nh
    cos_arr = np.cos(freqs).astype(np.float32).reshape(n_st, 128, rh)
    cos_arr = np.broadcast_to(cos_arr[:, :, None, :], (n_st, 128, nh, rh))
    cos_arr = np.ascontiguousarray(
        cos_arr.transpose(1, 0, 2, 3).reshape(128, n_st * nh * rh))
    sin_arr = np.sin(freqs).astype(np.float32).reshape(n_st, 128, rh)
    sin_arr = np.broadcast_to(sin_arr[:, :, None, :], (n_st, 128, nh, rh))
    sin_arr = np.ascontiguousarray(
        sin_arr.transpose(1, 0, 2, 3).reshape(128, n_st * nh * rh))

    km = np.arange(128)[:, None]
    qm = np.arange(qb)[None, :]
    mask_arr = np.concatenate(
        [(qm >= i * 128 + km).astype(np.float32) for i in range(ndiag)],
        axis=1)

    xa = np.ascontiguousarray(x[bi].T)
    xs_arr = np.ascontiguousarray(
        xa.reshape(n_kt, 128, n_st, 128).transpose(2, 1, 0, 3)
    ).reshape(n_st, 128, n_kt * 128)

    return {
        "xs": xs_arr.astype(npdt),
        "wqk": np.ascontiguousarray(wqk_arr).astype(npdt),
        "wv": np.ascontiguousarray(wv_arr).astype(npdt),
        "outw": outw_arr.astype(npdt),
        "cosR": cos_arr.astype(npdt),
        "sinR": sin_arr.astype(npdt),
        "masks": np.ascontiguousarray(mask_arr).astype(npdt),
        "bqk": np.ascontiguousarray(bqk_arr).astype(npdt),
        "bv": np.ascontiguousarray(bv_arr).astype(npdt),
    }


_CACHE = {}


def run_mha(cfg, x, wqkv_w, wqkv_b, out_w, out_b, trace=False):
    key = tuple(sorted(cfg.items()))
    if key not in _CACHE:
        _CACHE[key] = build_program(cfg)
    nc = _CACHE[key]
    in_maps = [
        prep_core_inputs(cfg, x, wqkv_w, wqkv_b, out_w, c)
        for c in range(N_CORES)
    ]
    res = run_bass_kernel_spmd(nc, in_maps, core_ids=list(range(N_CORES)),
                               trace=trace)
    d, s = cfg["d"], cfg["s"]
    y = np.zeros((B, s, d), np.float32)
    for bi in range(B):
        acc = np.zeros((d, s), np.float32)
        for c in range(bi * NBG, (bi + 1) * NBG):
            acc += res.results[c]["y"].astype(np.float32)
        y[bi] = acc.T + out_b[None, :]
    return y, res


def kernel(x, wqkv_w, wqkv_b, out_w, out_b):
    cfg = make_cfg(dt=os.environ.get("KMHA_DT", "bf16"))
    y, _ = run_mha(cfg, np.asarray(x, np.float32),
                   np.asarray(wqkv_w, np.float32),
                   np.asarray(wqkv_b, np.float32),
                   np.asarray(out_w, np.float32),
                   np.asarray(out_b, np.float32))
    return y


# revision 4
# speedup vs baseline: 1.1793x; 1.0102x over previous
"""MHA Trainium2 kernel v2: (batch x head-group) sharded across 8 NeuronCores.

Problem: B=2, S=2048, D=2560, H=32 heads, HD=80, partial rotary RD=32,
causal attention, fp32 reference; kernel computes in bf16 with f32 PSUM.

Core c handles batch c//4, heads (c%4)*8 .. +8.

Per-core phases:
  A1: q,k projection (x-tile stationary, weight moving) -> stage [tok, 1280],
      rope on DVE (3D strided APs batching all 8 heads), PE-transpose per head
      to qT/kT [80, s].
  A2: v projection -> stage_v [tok, 8*(80+1)] resident; per-head col 80 is a
      bias-only "ones" channel so PV yields the softmax denominator.
  C:  per (head, q-block): scoresT [k,q] on PE, exp on ACT (f32 PSUM -> bf16),
      causal mask multiply on diagonal tiles (DVE), PV accumulate [81, qb],
      reciprocal of row 80 + PE-broadcast + DVE multiply -> ctxT [80, s].
      ctxT repacked into [128, s] tiles via SBUF->SBUF DMA per head.
  D:  out-proj yT [d, s] partial (K=128 packed ctx), DMA out bf16.
  host: sum 4 partials per batch in f32, transpose, add out bias.
"""
import sys
import os

sys.path.insert(0, "/opt/trn_rl_repo")

import numpy as np
from contextlib import ExitStack

import concourse.bacc as bacc
import concourse.tile as tile
import concourse.mybir as mybir
from concourse.bass_utils import run_bass_kernel_spmd
from concourse.masks import make_identity

F32 = mybir.dt.float32
F32R = mybir.dt.float32r
BF16 = mybir.dt.bfloat16

B, S, D = 2, 2048, 2560
H, HD = 32, 80
RD = 32
ROPE_BASE = 10000.0
N_CORES = 8
NBG = 4  # cores per batch


def make_cfg(s=S, d=D, nh=H // NBG, hd=HD, rd=RD, qb=512, dt="bf16"):
    cfg = dict(s=s, d=d, nh=nh, hd=hd, rd=rd, qb=qb, dt=dt)
    cfg["n_st"] = s // 128
    cfg["n_kt"] = d // 128
    cfg["d_aug"] = cfg["n_kt"] * 128
    cfg["nqb"] = s // qb
    cfg["ndiag"] = qb // 128
    cfg["n_dt"] = d // 128
    cfg["jqk"] = 2 * nh * hd          # 1280
    cfg["vw"] = ((hd + 31) // 32) * 32 + 1  # 97: ones channel at 32-aligned col
    cfg["jv"] = nh * cfg["vw"]              # 776
    cfg["n_ct"] = (nh * hd) // 128    # 5 packed ctx tiles
    return cfg


def _dt(cfg):
    return {"bf16": BF16, "f32": F32, "f32r": F32R}[cfg["dt"]]


def _groups(total, step=512):
    out = []
    a = 0
    while a < total:
        out.append((a, min(a + step, total)))
        a += step
    return out


def build_program(cfg):
    s, d, nh, hd, rd = cfg["s"], cfg["d"], cfg["nh"], cfg["hd"], cfg["rd"]
    qb, n_st, n_kt = cfg["qb"], cfg["n_st"], cfg["n_kt"]
    nqb, ndiag, n_dt = cfg["nqb"], cfg["ndiag"], cfg["n_dt"]
    jqk, vw, jv, n_ct = cfg["jqk"], cfg["vw"], cfg["jv"], cfg["n_ct"]
    DT = _dt(cfg)
    rh = rd // 2

    nc = bacc.Bacc(None, debug=False)

    xs_d = nc.declare_dram_parameter("xs", [n_st, 128, n_kt * 128], DT,
                                     isOutput=False)
    wqk_d = nc.declare_dram_parameter("wqk", [n_kt, 128, jqk], DT,
                                      isOutput=False)
    wv_d = nc.declare_dram_parameter("wv", [n_kt, 128, jv], DT, isOutput=False)
    outw_d = nc.declare_dram_parameter("outw", [n_ct, 128, d], DT,
                                       isOutput=False)
    cos_d = nc.declare_dram_parameter("cosR", [128, n_st * nh * rh], DT,
                                      isOutput=False)
    sin_d = nc.declare_dram_parameter("sinR", [128, n_st * nh * rh], DT,
                                      isOutput=False)
    mask_d = nc.declare_dram_parameter("masks", [128, ndiag * qb], DT,
                                       isOutput=False)
    bqk_d = nc.declare_dram_parameter("bqk", [128, jqk], DT, isOutput=False)
    bv_d = nc.declare_dram_parameter("bv", [128, jv], DT, isOutput=False)
    y_d = nc.declare_dram_parameter("y", [d, s], DT, isOutput=True)

    with tile.TileContext(nc) as tc, ExitStack() as top:
        top.enter_context(
            nc.allow_low_precision(reason="intentional bf16 storage"))
        glob = top.enter_context(tc.tile_pool(name="glob", bufs=1))
        identf = glob.tile([128, 128], F32)
        make_identity(nc, identf)
        if DT is F32:
            ident = identf
        else:
            ident = glob.tile([128, 128], DT)
            nc.vector.tensor_copy(ident, identf)
        ones1f = glob.tile([1, hd], F32)
        nc.vector.memset(ones1f, 1.0)
        ones1 = glob.tile([1, hd], F32R)
        nc.vector.tensor_copy(ones1, ones1f)
        cosR = glob.tile([128, n_st * nh * rh], DT)
        sinR = glob.tile([128, n_st * nh * rh], DT)
        masks = glob.tile([128, ndiag * qb], DT)
        bqk = glob.tile([128, jqk], DT)
        bv = glob.tile([128, jv], DT)

        qt_pool = top.enter_context(tc.tile_pool(name="qt", bufs=1))
        qT = [qt_pool.tile([hd, s], DT, tag=f"q{h}", name=f"qT{h}")
              for h in range(nh)]
        kT = [qt_pool.tile([hd, s], DT, tag=f"k{h}", name=f"kT{h}")
              for h in range(nh)]
        vp = top.enter_context(tc.tile_pool(name="vp", bufs=1))
        vA = [vp.tile([128, jv], DT, tag=f"v{st}", name=f"vA{st}")
              for st in range(n_st)]

        # ---- phase A1: q,k projection + rope + transpose ----
        qk_groups = _groups(jqk)
        xsp = top.enter_context(tc.tile_pool(name="xsp", bufs=2))
        xs0 = xsp.tile([128, n_kt * 128], DT, tag="xs", name="xs_a1_0")
        nc.sync.dma_start(out=xs0, in_=xs_d[0])
        with ExitStack() as a1:
            wp = a1.enter_context(tc.tile_pool(name="w1", bufs=1))
            wqk = [wp.tile([128, jqk], DT, tag=f"w1_{kt}", name=f"wqk{kt}")
                   for kt in range(n_kt)]
            for kt in range(4):
                nc.sync.dma_start(out=wqk[kt], in_=wqk_d[kt])
            nc.sync.dma_start(out=cosR, in_=cos_d[:, :])
            nc.sync.dma_start(out=sinR, in_=sin_d[:, :])
            nc.sync.dma_start(out=masks, in_=mask_d[:, :])
            nc.sync.dma_start(out=bqk, in_=bqk_d[:, :])
            nc.sync.dma_start(out=bv, in_=bv_d[:, :])
            for kt in range(4, n_kt):
                nc.sync.dma_start(out=wqk[kt], in_=wqk_d[kt])
            stp = a1.enter_context(tc.tile_pool(name="stg1", bufs=2))
            psA = a1.enter_context(
                tc.tile_pool(name="psA", bufs=2, space="PSUM"))
            psT = a1.enter_context(
                tc.tile_pool(name="psT", bufs=2, space="PSUM"))
            rtp = a1.enter_context(tc.tile_pool(name="rt", bufs=2))
            for st in range(n_st):
                if st == 0:
                    xs = xs0
                else:
                    xs = xsp.tile([128, n_kt * 128], DT, tag="xs",
                                  name=f"xs_a1_{st}")
                    nc.sync.dma_start(out=xs, in_=xs_d[st])
                xs3 = xs.rearrange("p (t c) -> p t c", t=n_kt)
                ps = [psA.tile([128, g1 - g0], F32, tag=f"ps{gi}", name=f"psA{st}_{gi}")
                      for gi, (g0, g1) in enumerate(qk_groups)]
                for kt in range(n_kt):
                    for gi, (g0, g1) in enumerate(qk_groups):
                        nc.tensor.matmul(
                            ps[gi], xs3[:, kt, :], wqk[kt][:, g0:g1],
                            start=(kt == 0), stop=(kt == n_kt - 1))
                stage = stp.tile([128, jqk], DT, tag="stage")
                for gi, (g0, g1) in enumerate(qk_groups):
                    nc.vector.scalar_tensor_tensor(
                        out=stage[:, g0:g1], in0=ps[gi], scalar=1.0,
                        in1=bqk[:, g0:g1], op0=mybir.AluOpType.mult,
                        op1=mybir.AluOpType.add)
                # rope: all nh heads per op via strided 3D views
                cN = cosR[:, st * nh * rh:(st + 1) * nh * rh] \
                    .rearrange("p (h c) -> p h c", h=nh)
                sN = sinR[:, st * nh * rh:(st + 1) * nh * rh] \
                    .rearrange("p (h c) -> p h c", h=nh)
                for qk in range(2):
                    blk = stage[:, qk * nh * hd:(qk + 1) * nh * hd] \
                        .rearrange("p (h c) -> p h c", h=nh)
                    t1 = blk[:, :, 0:rh]
                    t2 = blk[:, :, rh:rd]
                    ta = rtp.tile([128, nh, rh], F32, tag="ta")
                    tb = rtp.tile([128, nh, rh], F32, tag="tb")
                    tg = rtp.tile([128, nh, rh], F32, tag="tg")
                    td = rtp.tile([128, nh, rh], F32, tag="td")
                    nc.vector.tensor_mul(ta, t1, cN)
                    nc.vector.tensor_mul(tb, t2, sN)
                    nc.vector.tensor_mul(tg, t1, sN)
                    nc.vector.tensor_mul(td, t2, cN)
                    nc.vector.tensor_sub(t1, ta, tb)
                    nc.vector.tensor_add(t2, tg, td)
                for h in range(nh):
                    for qk, dstT in ((0, qT), (1, kT)):
                        pt = psT.tile([hd, 128], DT, tag="pt")
                        nc.tensor.transpose(
                            pt, stage[:, qk * nh * hd + h * hd:
                                      qk * nh * hd + (h + 1) * hd], ident)
                        nc.vector.tensor_copy(
                            dstT[h][:, st * 128:(st + 1) * 128], pt)

        # ---- phase A2: v projection (+ ones channel per head) ----
        v_groups = _groups(jv)
        with ExitStack() as a2:
            wp2 = a2.enter_context(tc.tile_pool(name="w2", bufs=1))
            wv = [wp2.tile([128, jv], DT, tag=f"w2_{kt}", name=f"wv{kt}")
                  for kt in range(n_kt)]
            for kt in range(n_kt):
                nc.sync.dma_start(out=wv[kt], in_=wv_d[kt])
            psA2 = a2.enter_context(
                tc.tile_pool(name="psA2", bufs=2, space="PSUM"))
            for st in range(n_st):
                xs = xsp.tile([128, n_kt * 128], DT, tag="xs",
                              name=f"xs_a2_{st}")
                nc.sync.dma_start(out=xs, in_=xs_d[st])
                xs3 = xs.rearrange("p (t c) -> p t c", t=n_kt)
                ps = [psA2.tile([128, g1 - g0], F32, tag=f"ps{gi}", name=f"psA2{st}_{gi}")
                      for gi, (g0, g1) in enumerate(v_groups)]
                for kt in range(n_kt):
                    for gi, (g0, g1) in enumerate(v_groups):
                        nc.tensor.matmul(
                            ps[gi], xs3[:, kt, :], wv[kt][:, g0:g1],
                            start=(kt == 0), stop=(kt == n_kt - 1))
                for gi, (g0, g1) in enumerate(v_groups):
                    nc.vector.scalar_tensor_tensor(
                        out=vA[st][:, g0:g1], in0=ps[gi], scalar=1.0,
                        in1=bv[:, g0:g1], op0=mybir.AluOpType.mult,
                        op1=mybir.AluOpType.add)

        # ---- phase C: attention ----
        ctx_pool = top.enter_context(tc.tile_pool(name="ctx", bufs=1))
        ctxT = [ctx_pool.tile([hd, s], DT, tag=f"c{h}", name=f"ctxT{h}")
                for h in range(nh)]
        ctxP = [ctx_pool.tile([128, s], DT, tag=f"cp{t}", name=f"ctxP{t}")
                for t in range(n_ct)]

        def repack(h, c0, c1):
            g0 = h * hd
            r = g0
            while r < g0 + hd:
                ct = r // 128
                r1 = min((ct + 1) * 128, g0 + hd)
                nc.sync.dma_start(
                    out=ctxP[ct][r - ct * 128:r1 - ct * 128, c0:c1],
                    in_=ctxT[h][r - g0:r1 - g0, c0:c1])
                r = r1

        op = top.enter_context(tc.tile_pool(name="ow", bufs=1))
        ow = [op.tile([128, d], DT, tag=f"o{t}", name=f"ow{t}")
              for t in range(n_ct)]
        for t in range(n_ct):
            nc.sync.dma_start(out=ow[t], in_=outw_d[t])

        with ExitStack() as cstk:
            pp = cstk.enter_context(tc.tile_pool(name="pT", bufs=3))
            rp2 = cstk.enter_context(tc.tile_pool(name="rr", bufs=1))
            psS = cstk.enter_context(
                tc.tile_pool(name="psS", bufs=2, space="PSUM"))
            psC = cstk.enter_context(
                tc.tile_pool(name="psC", bufs=1, space="PSUM"))
            psD = cstk.enter_context(
                tc.tile_pool(name="psD", bufs=2, space="PSUM"))
            stp2 = cstk.enter_context(tc.tile_pool(name="st2", bufs=2))

            def finalize(h, q0, pctx):
                # copy den + ctx out of PSUM first: releases the pctx bank in
                # ~0.7us so the next block's accumulation can start while the
                # recip/broadcast/normalize chain runs from SBUF.
                den = rp2.tile([1, qb], F32, tag="rd")
                nc.vector.tensor_copy(den, pctx[vw - 1:vw, :])
                ctx_s = rp2.tile([hd, qb], DT, tag="cs")
                nc.vector.tensor_copy(ctx_s, pctx[0:hd, :])
                rden = rp2.tile([1, qb], F32, tag="rdf")
                nc.vector.reciprocal_approx_fast(out=rden, in_=den)
                rdenr = rp2.tile([1, qb], F32R, tag="rdr")
                nc.vector.tensor_copy(rdenr, rden)
                pbc = psD.tile([hd, qb], F32, tag="ps")
                nc.tensor.matmul(pbc, ones1, rdenr, start=True, stop=True)
                rb = rp2.tile([hd, qb], DT, tag="rb")
                nc.vector.tensor_copy(rb, pbc)
                nc.vector.tensor_mul(
                    ctxT[h][:, q0 * qb:(q0 + 1) * qb], ctx_s, rb)
                repack(h, q0 * qb, (q0 + 1) * qb)

            # D-phase work units are drip-fed into the C instruction
            # stream as PE filler: C alone is ACT(exp)-bound, which idles
            # the PE long enough for HAM to re-throttle the clock.
            pending_d = []

            def emit_d_unit(dt_i, sb):
                psy = psD.tile([128, qb], F32, tag="ps",
                               name=f"psy{dt_i}_{sb}")
                for ct in range(n_ct):
                    nc.tensor.matmul(
                        psy, ow[ct][:, dt_i * 128:(dt_i + 1) * 128],
                        ctxP[ct][:, sb * qb:(sb + 1) * qb],
                        start=(ct == 0), stop=(ct == n_ct - 1))
                ystage = stp2.tile([128, qb], DT, tag="y",
                                   name=f"yst{dt_i}_{sb}")
                nc.vector.tensor_copy(ystage, psy)
                nc.sync.dma_start(
                    out=y_d[dt_i * 128:(dt_i + 1) * 128,
                            sb * qb:(sb + 1) * qb],
                    in_=ystage)

            # per head: two sweeps of q0-pairs; kt-outer within a sweep.
            # pctx[parity] lives in 1 PSUM bank each; pss2 [128, 2*qb]
            # spans 2 banks and holds both q0s of the sweep per kt.
            dstep = 0
            for sw in range(nqb // 2):
                q0s = (2 * sw, 2 * sw + 1)
                for h in range(nh):
                    pctxs = {
                        q0: psC.tile([vw, qb], F32, tag=f"pc{q0 % 2}",
                                     name=f"pctx{h}_{q0}")
                        for q0 in q0s
                    }
                    nkt_max = (q0s[1] + 1) * qb // 128
                    for kt in range(nkt_max):
                        act = [q0 for q0 in q0s
                               if kt < (q0 + 1) * qb // 128]
                        pss = {}
                        for ci, q0 in enumerate(act):
                            pss[q0] = psS.tile([128, qb], F32,
                                               tag=f"ss{q0 % 2}",
                                               name=f"pss{h}_{kt}_{q0}")
                            nc.tensor.matmul(
                                pss[q0],
                                kT[h][:, kt * 128:(kt + 1) * 128],
                                qT[h][:, q0 * qb:(q0 + 1) * qb],
                                start=True, stop=True)
                        pTs = {}
                        for ci, q0 in enumerate(act):
                            pTs[q0] = pp.tile([128, qb], DT, tag="p",
                                              name=f"pT{h}_{kt}_{q0}")
                            nc.scalar.activation(
                                pTs[q0], pss[q0],
                                mybir.ActivationFunctionType.Exp)
                            od = kt * 128 - q0 * qb
                            if od >= 0:
                                oi = od // 128
                                nc.vector.tensor_mul(
                                    pTs[q0], pTs[q0],
                                    masks[:, oi * qb:(oi + 1) * qb])
                        for ci, q0 in enumerate(act):
                            nkt_q = (q0 + 1) * qb // 128
                            nc.tensor.matmul(
                                pctxs[q0], vA[kt][:, h * vw:(h + 1) * vw],
                                pTs[q0],
                                start=(kt == 0), stop=(kt == nkt_q - 1))
                            if kt == nkt_q - 1:
                                finalize(h, q0, pctxs[q0])
                        dstep += 1
                        if pending_d and dstep % 2 == 0:
                            emit_d_unit(*pending_d.pop(0))
                # this sweep's out-projection units become ready now
                pending_d.extend(
                    (dt_i, sb) for dt_i in range(n_dt) for sb in q0s)
            while pending_d:
                emit_d_unit(*pending_d.pop(0))

    nc.finalize()
    return nc


def prep_core_inputs(cfg, x, wqkv_w, wqkv_b, out_w, core):
    s, d, nh, hd, rd = cfg["s"], cfg["d"], cfg["nh"], cfg["hd"], cfg["rd"]
    qb, n_st, n_kt, d_aug = cfg["qb"], cfg["n_st"], cfg["n_kt"], cfg["d_aug"]
    ndiag, jqk, vw, jv = cfg["ndiag"], cfg["jqk"], cfg["vw"], cfg["jv"]
    n_ct = cfg["n_ct"]
    rh = rd // 2
    npdt = mybir.dt.np(_dt(cfg))

    bi = core // NBG
    hg = core % NBG
    heads = range(hg * nh, (hg + 1) * nh)
    rows = np.concatenate([np.arange(h * hd, (h + 1) * hd) for h in heads])
    scale = np.float32(1.0 / np.sqrt(hd))

    wq = wqkv_w[rows, :]
    bq = wqkv_b[rows]
    wk = wqkv_w[d + rows, :] * scale
    bk = wqkv_b[d + rows] * scale
    wv = wqkv_w[2 * d + rows, :]
    bv = wqkv_b[2 * d + rows]

    def wt_tiles(w):
        return np.ascontiguousarray(w.T).reshape(n_kt, 128, w.shape[0])

    wqk_arr = np.concatenate([wt_tiles(wq), wt_tiles(wk)], axis=2)
    bqk_arr = np.broadcast_to(
        np.concatenate([bq, bk])[None, :], (128, jqk))

    # v: per-head block of vw cols; col vw-1 is the "ones" channel whose
    # weights are zero and bias is 1 (softmax denominator trick)
    wva = np.zeros((d, jv), np.float32)
    bva = np.zeros((jv,), np.float32)
    for h in range(nh):
        wva[:, h * vw:h * vw + hd] = wv[h * hd:(h + 1) * hd].T
        bva[h * vw:h * vw + hd] = bv[h * hd:(h + 1) * hd]
        bva[h * vw + vw - 1] = 1.0
    wv_arr = wva.reshape(n_kt, 128, jv)
    bv_arr = np.broadcast_to(bva[None, :], (128, jv))

    outw_arr = np.ascontiguousarray(
        out_w[:, rows].T.reshape(n_ct, 128, d))

    inv_freq = 1.0 / (ROPE_BASE ** (np.arange(0, rd, 2, dtype=np.float32) / rd))
    t = np.arange(s, dtype=np.float32)
    freqs = np.outer(t, inv_freq)  # [s, rh]
    # [128, n_st, nh, rh]: value depends on (token=st*128+p, freq i); repl. nh
    cos_arr = np.cos(freqs).astype(np.float32).reshape(n_st, 128, rh)
    cos_arr = np.broadcast_to(cos_arr[:, :, None, :], (n_st, 128, nh, rh))
    cos_arr = np.ascontiguousarray(
        cos_arr.transpose(1, 0, 2, 3).reshape(128, n_st * nh * rh))
    sin_arr = np.sin(freqs).astype(np.float32).reshape(n_st, 128, rh)
    sin_arr = np.broadcast_to(sin_arr[:, :, None, :], (n_st, 128, nh, rh))
    sin_arr = np.ascontiguousarray(
        sin_arr.transpose(1, 0, 2, 3).reshape(128, n_st * nh * rh))

    km = np.arange(128)[:, None]
    qm = np.arange(qb)[None, :]
    mask_arr = np.concatenate(
        [(qm >= i * 128 + km).astype(np.float32) for i in range(ndiag)],
        axis=1)

    xa = np.ascontiguousarray(x[bi].T)
    xs_arr = np.ascontiguousarray(
        xa.reshape(n_kt, 128, n_st, 128).transpose(2, 1, 0, 3)
    ).reshape(n_st, 128, n_kt * 128)

    return {
        "xs": xs_arr.astype(npdt),
        "wqk": np.ascontiguousarray(wqk_arr).astype(npdt),
        "wv": np.ascontiguousarray(wv_arr).astype(npdt),
        "outw": outw_arr.astype(npdt),
        "cosR": cos_arr.astype(npdt),
        "sinR": sin_arr.astype(npdt),
        "masks": np.ascontiguousarray(mask_arr).astype(npdt),
        "bqk": np.ascontiguousarray(bqk_arr).astype(npdt),
        "bv": np.ascontiguousarray(bv_arr).astype(npdt),
    }


_CACHE = {}


def run_mha(cfg, x, wqkv_w, wqkv_b, out_w, out_b, trace=False):
    key = tuple(sorted(cfg.items()))
    if key not in _CACHE:
        _CACHE[key] = build_program(cfg)
    nc = _CACHE[key]
    in_maps = [
        prep_core_inputs(cfg, x, wqkv_w, wqkv_b, out_w, c)
        for c in range(N_CORES)
    ]
    res = run_bass_kernel_spmd(nc, in_maps, core_ids=list(range(N_CORES)),
                               trace=trace)
    d, s = cfg["d"], cfg["s"]
    y = np.zeros((B, s, d), np.float32)
    for bi in range(B):
        acc = np.zeros((d, s), np.float32)
        for c in range(bi * NBG, (bi + 1) * NBG):
            acc += res.results[c]["y"].astype(np.float32)
        y[bi] = acc.T + out_b[None, :]
    return y, res


def kernel(x, wqkv_w, wqkv_b, out_w, out_b):
    cfg = make_cfg(dt=os.environ.get("KMHA_DT", "bf16"))
    y, _ = run_mha(cfg, np.asarray(x, np.float32),
                   np.asarray(wqkv_w, np.float32),
                   np.asarray(wqkv_b, np.float32),
                   np.asarray(out_w, np.float32),
                   np.asarray(out_b, np.float32))
    return y


# revision 5
# speedup vs baseline: 1.2182x; 1.0329x over previous
"""MHA Trainium2 kernel v2: (batch x head-group) sharded across 8 NeuronCores.

Problem: B=2, S=2048, D=2560, H=32 heads, HD=80, partial rotary RD=32,
causal attention, fp32 reference; kernel computes in bf16 with f32 PSUM.

Core c handles batch c//4, heads (c%4)*8 .. +8.

Per-core phases:
  A1: q,k projection (x-tile stationary, weight moving) -> stage [tok, 1280],
      rope on DVE (3D strided APs batching all 8 heads), PE-transpose per head
      to qT/kT [80, s].
  A2: v projection -> stage_v [tok, 8*(80+1)] resident; per-head col 80 is a
      bias-only "ones" channel so PV yields the softmax denominator.
  C:  per (head, q-block): scoresT [k,q] on PE, exp on ACT (f32 PSUM -> bf16),
      causal mask multiply on diagonal tiles (DVE), PV accumulate [81, qb],
      reciprocal of row 80 + PE-broadcast + DVE multiply -> ctxT [80, s].
      ctxT repacked into [128, s] tiles via SBUF->SBUF DMA per head.
  D:  out-proj yT [d, s] partial (K=128 packed ctx), DMA out bf16.
  host: sum 4 partials per batch in f32, transpose, add out bias.
"""
import sys
import os

sys.path.insert(0, "/opt/trn_rl_repo")

import numpy as np
from contextlib import ExitStack

import concourse.bacc as bacc
import concourse.tile as tile
import concourse.mybir as mybir
from concourse.bass_utils import run_bass_kernel_spmd
from concourse.masks import make_identity

F32 = mybir.dt.float32
F32R = mybir.dt.float32r
BF16 = mybir.dt.bfloat16

B, S, D = 2, 2048, 2560
H, HD = 32, 80
RD = 32
ROPE_BASE = 10000.0
N_CORES = 8
NBG = 4  # cores per batch


def make_cfg(s=S, d=D, nh=H // NBG, hd=HD, rd=RD, qb=512, dt="bf16"):
    cfg = dict(s=s, d=d, nh=nh, hd=hd, rd=rd, qb=qb, dt=dt)
    cfg["n_st"] = s // 128
    cfg["n_kt"] = d // 128
    cfg["d_aug"] = cfg["n_kt"] * 128
    cfg["nqb"] = s // qb
    cfg["ndiag"] = qb // 128
    cfg["n_dt"] = d // 128
    cfg["jqk"] = 2 * nh * hd          # 1280
    cfg["vw"] = ((hd + 31) // 32) * 32 + 1  # 97: ones channel at 32-aligned col
    cfg["jv"] = nh * cfg["vw"]              # 776 (vA layout, incl. pads)
    cfg["jvc"] = nh * (hd + 1)              # 648 (compact weight cols)
    cfg["n_ct"] = (nh * hd) // 128    # 5 packed ctx tiles
    return cfg


def _dt(cfg):
    return {"bf16": BF16, "f32": F32, "f32r": F32R}[cfg["dt"]]


def _groups(total, step=512):
    out = []
    a = 0
    while a < total:
        out.append((a, min(a + step, total)))
        a += step
    return out


def build_program(cfg):
    s, d, nh, hd, rd = cfg["s"], cfg["d"], cfg["nh"], cfg["hd"], cfg["rd"]
    qb, n_st, n_kt = cfg["qb"], cfg["n_st"], cfg["n_kt"]
    nqb, ndiag, n_dt = cfg["nqb"], cfg["ndiag"], cfg["n_dt"]
    jqk, vw, jv, n_ct = cfg["jqk"], cfg["vw"], cfg["jv"], cfg["n_ct"]
    jvc = cfg["jvc"]
    DT = _dt(cfg)
    rh = rd // 2

    nc = bacc.Bacc(None, debug=False)

    xs_d = nc.declare_dram_parameter("xs", [n_st, 128, n_kt * 128], DT,
                                     isOutput=False)
    wqk_d = nc.declare_dram_parameter("wqk", [n_kt, 128, jqk], DT,
                                      isOutput=False)
    wv_d = nc.declare_dram_parameter("wv", [n_kt, 128, jvc], DT,
                                     isOutput=False)
    outw_d = nc.declare_dram_parameter("outw", [n_ct, 128, d], DT,
                                       isOutput=False)
    cos_d = nc.declare_dram_parameter("cosR", [128, n_st * nh * rh], DT,
                                      isOutput=False)
    sin_d = nc.declare_dram_parameter("sinR", [128, n_st * nh * rh], DT,
                                      isOutput=False)
    mask_d = nc.declare_dram_parameter("masks", [128, ndiag * qb], DT,
                                       isOutput=False)
    bqk_d = nc.declare_dram_parameter("bqk", [128, jqk], DT, isOutput=False)
    bv_d = nc.declare_dram_parameter("bv", [128, jvc], DT, isOutput=False)
    y_d = nc.declare_dram_parameter("y", [d, s], DT, isOutput=True)

    with tile.TileContext(nc) as tc, ExitStack() as top:
        top.enter_context(
            nc.allow_low_precision(reason="intentional bf16 storage"))
        glob = top.enter_context(tc.tile_pool(name="glob", bufs=1))
        identf = glob.tile([128, 128], F32)
        make_identity(nc, identf)
        if DT is F32:
            ident = identf
        else:
            ident = glob.tile([128, 128], DT)
            nc.vector.tensor_copy(ident, identf)
        ones1f = glob.tile([1, hd], F32)
        nc.vector.memset(ones1f, 1.0)
        ones1 = glob.tile([1, hd], F32R)
        nc.vector.tensor_copy(ones1, ones1f)
        cosR = glob.tile([128, n_st * nh * rh], DT)
        sinR = glob.tile([128, n_st * nh * rh], DT)
        masks = glob.tile([128, ndiag * qb], DT)
        bqk = glob.tile([128, jqk], DT)
        bv = glob.tile([128, jvc], DT)

        qt_pool = top.enter_context(tc.tile_pool(name="qt", bufs=1))
        qT = [qt_pool.tile([hd, s], DT, tag=f"q{h}", name=f"qT{h}")
              for h in range(nh)]
        kT = [qt_pool.tile([hd, s], DT, tag=f"k{h}", name=f"kT{h}")
              for h in range(nh)]
        vp = top.enter_context(tc.tile_pool(name="vp", bufs=1))
        vA = [vp.tile([128, jv], DT, tag=f"v{st}", name=f"vA{st}")
              for st in range(n_st)]
        for st in range(n_st):
            nc.vector.memset(vA[st], 0.0)

        # ---- phase A1: q,k projection + rope + transpose ----
        qk_groups = _groups(jqk)
        xsp = top.enter_context(tc.tile_pool(name="xsp", bufs=2))
        xs0 = xsp.tile([128, n_kt * 128], DT, tag="xs", name="xs_a1_0")
        nc.sync.dma_start(out=xs0, in_=xs_d[0])
        with ExitStack() as a1:
            wp = a1.enter_context(tc.tile_pool(name="w1", bufs=1))
            wqk = [wp.tile([128, jqk], DT, tag=f"w1_{kt}", name=f"wqk{kt}")
                   for kt in range(n_kt)]
            for kt in range(4):
                nc.sync.dma_start(out=wqk[kt], in_=wqk_d[kt])
            nc.sync.dma_start(out=cosR, in_=cos_d[:, :])
            nc.sync.dma_start(out=sinR, in_=sin_d[:, :])
            nc.sync.dma_start(out=masks, in_=mask_d[:, :])
            nc.sync.dma_start(out=bqk, in_=bqk_d[:, :])
            nc.sync.dma_start(out=bv, in_=bv_d[:, :])
            for kt in range(4, n_kt):
                nc.sync.dma_start(out=wqk[kt], in_=wqk_d[kt])
            stp = a1.enter_context(tc.tile_pool(name="stg1", bufs=2))
            psA = a1.enter_context(
                tc.tile_pool(name="psA", bufs=2, space="PSUM"))
            psT = a1.enter_context(
                tc.tile_pool(name="psT", bufs=2, space="PSUM"))
            rtp = a1.enter_context(tc.tile_pool(name="rt", bufs=2))
            for st in range(n_st):
                if st == 0:
                    xs = xs0
                else:
                    xs = xsp.tile([128, n_kt * 128], DT, tag="xs",
                                  name=f"xs_a1_{st}")
                    nc.sync.dma_start(out=xs, in_=xs_d[st])
                xs3 = xs.rearrange("p (t c) -> p t c", t=n_kt)
                ps = [psA.tile([128, g1 - g0], F32, tag=f"ps{gi}", name=f"psA{st}_{gi}")
                      for gi, (g0, g1) in enumerate(qk_groups)]
                for kt in range(n_kt):
                    for gi, (g0, g1) in enumerate(qk_groups):
                        nc.tensor.matmul(
                            ps[gi], xs3[:, kt, :], wqk[kt][:, g0:g1],
                            start=(kt == 0), stop=(kt == n_kt - 1))
                stage = stp.tile([128, jqk], DT, tag="stage")
                for gi, (g0, g1) in enumerate(qk_groups):
                    nc.vector.scalar_tensor_tensor(
                        out=stage[:, g0:g1], in0=ps[gi], scalar=1.0,
                        in1=bqk[:, g0:g1], op0=mybir.AluOpType.mult,
                        op1=mybir.AluOpType.add)
                # rope: all nh heads per op via strided 3D views
                cN = cosR[:, st * nh * rh:(st + 1) * nh * rh] \
                    .rearrange("p (h c) -> p h c", h=nh)
                sN = sinR[:, st * nh * rh:(st + 1) * nh * rh] \
                    .rearrange("p (h c) -> p h c", h=nh)
                for qk in range(2):
                    blk = stage[:, qk * nh * hd:(qk + 1) * nh * hd] \
                        .rearrange("p (h c) -> p h c", h=nh)
                    t1 = blk[:, :, 0:rh]
                    t2 = blk[:, :, rh:rd]
                    ta = rtp.tile([128, nh, rh], F32, tag="ta")
                    tb = rtp.tile([128, nh, rh], F32, tag="tb")
                    tg = rtp.tile([128, nh, rh], F32, tag="tg")
                    td = rtp.tile([128, nh, rh], F32, tag="td")
                    nc.vector.tensor_mul(ta, t1, cN)
                    nc.vector.tensor_mul(tb, t2, sN)
                    nc.vector.tensor_mul(tg, t1, sN)
                    nc.vector.tensor_mul(td, t2, cN)
                    nc.vector.tensor_sub(t1, ta, tb)
                    nc.vector.tensor_add(t2, tg, td)
                for h in range(nh):
                    for qk, dstT in ((0, qT), (1, kT)):
                        pt = psT.tile([hd, 128], DT, tag="pt")
                        nc.tensor.transpose(
                            pt, stage[:, qk * nh * hd + h * hd:
                                      qk * nh * hd + (h + 1) * hd], ident)
                        nc.vector.tensor_copy(
                            dstT[h][:, st * 128:(st + 1) * 128], pt)

        # ---- phase A2: v projection (+ ones channel per head) ----
        # weights are compact (81 cols/head); copies re-stride into the
        # padded vA layout with the ones channel landing at col 96.
        hw1 = hd + 1
        v_groups = [(0, 6 * hw1), (6 * hw1, nh * hw1)]  # head-aligned
        with ExitStack() as a2:
            wp2 = a2.enter_context(tc.tile_pool(name="w2", bufs=1))
            wv = [wp2.tile([128, jvc], DT, tag=f"w2_{kt}", name=f"wv{kt}")
                  for kt in range(n_kt)]
            for kt in range(n_kt):
                nc.sync.dma_start(out=wv[kt], in_=wv_d[kt])
            psA2 = a2.enter_context(
                tc.tile_pool(name="psA2", bufs=2, space="PSUM"))
            for st in range(n_st):
                xs = xsp.tile([128, n_kt * 128], DT, tag="xs",
                              name=f"xs_a2_{st}")
                nc.sync.dma_start(out=xs, in_=xs_d[st])
                xs3 = xs.rearrange("p (t c) -> p t c", t=n_kt)
                ps = [psA2.tile([128, g1 - g0], F32, tag=f"ps{gi}", name=f"psA2{st}_{gi}")
                      for gi, (g0, g1) in enumerate(v_groups)]
                for kt in range(n_kt):
                    for gi, (g0, g1) in enumerate(v_groups):
                        nc.tensor.matmul(
                            ps[gi], xs3[:, kt, :], wv[kt][:, g0:g1],
                            start=(kt == 0), stop=(kt == n_kt - 1))
                vA3 = vA[st].rearrange("p (h c) -> p h c", h=nh)
                for gi, (g0, g1) in enumerate(v_groups):
                    h0, h1 = g0 // hw1, g1 // hw1
                    ps3 = ps[gi].rearrange("p (h c) -> p h c", h=h1 - h0)
                    bv3 = bv[:, g0:g1].rearrange("p (h c) -> p h c",
                                                 h=h1 - h0)
                    nc.vector.scalar_tensor_tensor(
                        out=vA3[:, h0:h1, 0:hd], in0=ps3[:, :, 0:hd],
                        scalar=1.0, in1=bv3[:, :, 0:hd],
                        op0=mybir.AluOpType.mult, op1=mybir.AluOpType.add)
                    nc.vector.scalar_tensor_tensor(
                        out=vA3[:, h0:h1, vw - 1:vw],
                        in0=ps3[:, :, hd:hd + 1], scalar=1.0,
                        in1=bv3[:, :, hd:hd + 1],
                        op0=mybir.AluOpType.mult, op1=mybir.AluOpType.add)

        # ---- phase C: attention ----
        ctx_pool = top.enter_context(tc.tile_pool(name="ctx", bufs=1))
        ctxP = [ctx_pool.tile([128, s], DT, tag=f"cp{t}", name=f"ctxP{t}")
                for t in range(n_ct)]

        def repack(h, c0, c1, src_tile):
            g0 = h * hd
            r = g0
            while r < g0 + hd:
                ct = r // 128
                r1 = min((ct + 1) * 128, g0 + hd)
                nc.sync.dma_start(
                    out=ctxP[ct][r - ct * 128:r1 - ct * 128, c0:c1],
                    in_=src_tile[r - g0:r1 - g0, :])
                r = r1

        op = top.enter_context(tc.tile_pool(name="ow", bufs=1))
        ow = [op.tile([128, d], DT, tag=f"o{t}", name=f"ow{t}")
              for t in range(n_ct)]
        for t in range(n_ct):
            nc.sync.dma_start(out=ow[t], in_=outw_d[t])

        with ExitStack() as cstk:
            pp = cstk.enter_context(tc.tile_pool(name="pT", bufs=3))
            rp2 = cstk.enter_context(tc.tile_pool(name="rr", bufs=2))
            psS = cstk.enter_context(
                tc.tile_pool(name="psS", bufs=2, space="PSUM"))
            psC = cstk.enter_context(
                tc.tile_pool(name="psC", bufs=1, space="PSUM"))
            psD = cstk.enter_context(
                tc.tile_pool(name="psD", bufs=2, space="PSUM"))
            stp2 = cstk.enter_context(tc.tile_pool(name="st2", bufs=2))

            def finalize(h, q0, pctx):
                # copy den + ctx out of PSUM first: releases the pctx bank in
                # ~0.7us so the next block's accumulation can start while the
                # recip/broadcast/normalize chain runs from SBUF.
                den = rp2.tile([1, qb], F32, tag="rd")
                nc.vector.tensor_copy(den, pctx[vw - 1:vw, :])
                ctx_s = rp2.tile([hd, qb], DT, tag="cs")
                nc.vector.tensor_copy(ctx_s, pctx[0:hd, :])
                rden = rp2.tile([1, qb], F32, tag="rdf")
                nc.vector.reciprocal_approx_fast(out=rden, in_=den)
                rdenr = rp2.tile([1, qb], F32R, tag="rdr")
                nc.vector.tensor_copy(rdenr, rden)
                pbc = psD.tile([hd, qb], F32, tag="ps")
                nc.tensor.matmul(pbc, ones1, rdenr, start=True, stop=True)
                rb = rp2.tile([hd, qb], DT, tag="rb")
                nc.vector.tensor_copy(rb, pbc)
                cts = rp2.tile([hd, qb], DT, tag="ctso", name=f"cts{h}_{q0}")
                nc.vector.tensor_mul(cts, ctx_s, rb)
                repack(h, q0 * qb, (q0 + 1) * qb, cts)

            # D-phase work units are drip-fed into the C instruction
            # stream as PE filler: C alone is ACT(exp)-bound, which idles
            # the PE long enough for HAM to re-throttle the clock.
            pending_d = []

            def emit_d_unit(dt_i, sb):
                psy = psD.tile([128, qb], F32, tag="ps",
                               name=f"psy{dt_i}_{sb}")
                for ct in range(n_ct):
                    nc.tensor.matmul(
                        psy, ow[ct][:, dt_i * 128:(dt_i + 1) * 128],
                        ctxP[ct][:, sb * qb:(sb + 1) * qb],
                        start=(ct == 0), stop=(ct == n_ct - 1))
                ystage = stp2.tile([128, qb], DT, tag="y",
                                   name=f"yst{dt_i}_{sb}")
                nc.vector.tensor_copy(ystage, psy)
                nc.sync.dma_start(
                    out=y_d[dt_i * 128:(dt_i + 1) * 128,
                            sb * qb:(sb + 1) * qb],
                    in_=ystage)

            # per head: two sweeps of q0-pairs; kt-outer within a sweep.
            # pctx[parity] lives in 1 PSUM bank each; pss2 [128, 2*qb]
            # spans 2 banks and holds both q0s of the sweep per kt.
            dstep = 0
            for sw in range(nqb // 2):
                q0s = (2 * sw, 2 * sw + 1)
                for h in range(nh):
                    pctxs = {
                        q0: psC.tile([vw, qb], F32, tag=f"pc{q0 % 2}",
                                     name=f"pctx{h}_{q0}")
                        for q0 in q0s
                    }
                    nkt_max = (q0s[1] + 1) * qb // 128
                    for kt in range(nkt_max):
                        act = [q0 for q0 in q0s
                               if kt < (q0 + 1) * qb // 128]
                        pss = {}
                        for ci, q0 in enumerate(act):
                            pss[q0] = psS.tile([128, qb], F32,
                                               tag=f"ss{q0 % 2}",
                                               name=f"pss{h}_{kt}_{q0}")
                            nc.tensor.matmul(
                                pss[q0],
                                kT[h][:, kt * 128:(kt + 1) * 128],
                                qT[h][:, q0 * qb:(q0 + 1) * qb],
                                start=True, stop=True)
                        pTs = {}
                        for ci, q0 in enumerate(act):
                            pTs[q0] = pp.tile([128, qb], DT, tag="p",
                                              name=f"pT{h}_{kt}_{q0}")
                            nc.scalar.activation(
                                pTs[q0], pss[q0],
                                mybir.ActivationFunctionType.Exp)
                            od = kt * 128 - q0 * qb
                            if od >= 0:
                                oi = od // 128
                                nc.vector.tensor_mul(
                                    pTs[q0], pTs[q0],
                                    masks[:, oi * qb:(oi + 1) * qb])
                        for ci, q0 in enumerate(act):
                            nkt_q = (q0 + 1) * qb // 128
                            nc.tensor.matmul(
                                pctxs[q0], vA[kt][:, h * vw:(h + 1) * vw],
                                pTs[q0],
                                start=(kt == 0), stop=(kt == nkt_q - 1))
                            if kt == nkt_q - 1:
                                finalize(h, q0, pctxs[q0])
                        dstep += 1
                        if pending_d and dstep % 2 == 0:
                            emit_d_unit(*pending_d.pop(0))
                # this sweep's out-projection units become ready now
                pending_d.extend(
                    (dt_i, sb) for dt_i in range(n_dt) for sb in q0s)
            while pending_d:
                emit_d_unit(*pending_d.pop(0))

    nc.finalize()
    return nc


def prep_core_inputs(cfg, x, wqkv_w, wqkv_b, out_w, core):
    s, d, nh, hd, rd = cfg["s"], cfg["d"], cfg["nh"], cfg["hd"], cfg["rd"]
    qb, n_st, n_kt, d_aug = cfg["qb"], cfg["n_st"], cfg["n_kt"], cfg["d_aug"]
    ndiag, jqk, vw, jv = cfg["ndiag"], cfg["jqk"], cfg["vw"], cfg["jv"]
    n_ct = cfg["n_ct"]
    rh = rd // 2
    npdt = mybir.dt.np(_dt(cfg))

    bi = core // NBG
    hg = core % NBG
    heads = range(hg * nh, (hg + 1) * nh)
    rows = np.concatenate([np.arange(h * hd, (h + 1) * hd) for h in heads])
    scale = np.float32(1.0 / np.sqrt(hd))

    wq = wqkv_w[rows, :]
    bq = wqkv_b[rows]
    wk = wqkv_w[d + rows, :] * scale
    bk = wqkv_b[d + rows] * scale
    wv = wqkv_w[2 * d + rows, :]
    bv = wqkv_b[2 * d + rows]

    def wt_tiles(w):
        return np.ascontiguousarray(w.T).reshape(n_kt, 128, w.shape[0])

    wqk_arr = np.concatenate([wt_tiles(wq), wt_tiles(wk)], axis=2)
    bqk_arr = np.broadcast_to(
        np.concatenate([bq, bk])[None, :], (128, jqk))

    # v: compact 81 cols per head (80 weights + ones channel with zero
    # weights and bias 1); the device copies re-stride into the vA layout.
    jvc = cfg["jvc"]
    hw1 = hd + 1
    wva = np.zeros((d, jvc), np.float32)
    bva = np.zeros((jvc,), np.float32)
    for h in range(nh):
        wva[:, h * hw1:h * hw1 + hd] = wv[h * hd:(h + 1) * hd].T
        bva[h * hw1:h * hw1 + hd] = bv[h * hd:(h + 1) * hd]
        bva[h * hw1 + hd] = 1.0
    wv_arr = wva.reshape(n_kt, 128, jvc)
    bv_arr = np.broadcast_to(bva[None, :], (128, jvc))

    outw_arr = np.ascontiguousarray(
        out_w[:, rows].T.reshape(n_ct, 128, d))

    inv_freq = 1.0 / (ROPE_BASE ** (np.arange(0, rd, 2, dtype=np.float32) / rd))
    t = np.arange(s, dtype=np.float32)
    freqs = np.outer(t, inv_freq)  # [s, rh]
    # [128, n_st, nh, rh]: value depends on (token=st*128+p, freq i); repl. nh
    cos_arr = np.cos(freqs).astype(np.float32).reshape(n_st, 128, rh)
    cos_arr = np.broadcast_to(cos_arr[:, :, None, :], (n_st, 128, nh, rh))
    cos_arr = np.ascontiguousarray(
        cos_arr.transpose(1, 0, 2, 3).reshape(128, n_st * nh * rh))
    sin_arr = np.sin(freqs).astype(np.float32).reshape(n_st, 128, rh)
    sin_arr = np.broadcast_to(sin_arr[:, :, None, :], (n_st, 128, nh, rh))
    sin_arr = np.ascontiguousarray(
        sin_arr.transpose(1, 0, 2, 3).reshape(128, n_st * nh * rh))

    km = np.arange(128)[:, None]
    qm = np.arange(qb)[None, :]
    mask_arr = np.concatenate(
        [(qm >= i * 128 + km).astype(np.float32) for i in range(ndiag)],
        axis=1)

    xa = np.ascontiguousarray(x[bi].T)
    xs_arr = np.ascontiguousarray(
        xa.reshape(n_kt, 128, n_st, 128).transpose(2, 1, 0, 3)
    ).reshape(n_st, 128, n_kt * 128)

    return {
        "xs": xs_arr.astype(npdt),
        "wqk": np.ascontiguousarray(wqk_arr).astype(npdt),
        "wv": np.ascontiguousarray(wv_arr).astype(npdt),
        "outw": outw_arr.astype(npdt),
        "cosR": cos_arr.astype(npdt),
        "sinR": sin_arr.astype(npdt),
        "masks": np.ascontiguousarray(mask_arr).astype(npdt),
        "bqk": np.ascontiguousarray(bqk_arr).astype(npdt),
        "bv": np.ascontiguousarray(bv_arr).astype(npdt),
    }


_CACHE = {}


def run_mha(cfg, x, wqkv_w, wqkv_b, out_w, out_b, trace=False):
    key = tuple(sorted(cfg.items()))
    if key not in _CACHE:
        _CACHE[key] = build_program(cfg)
    nc = _CACHE[key]
    in_maps = [
        prep_core_inputs(cfg, x, wqkv_w, wqkv_b, out_w, c)
        for c in range(N_CORES)
    ]
    res = run_bass_kernel_spmd(nc, in_maps, core_ids=list(range(N_CORES)),
                               trace=trace)
    d, s = cfg["d"], cfg["s"]
    y = np.zeros((B, s, d), np.float32)
    for bi in range(B):
        acc = np.zeros((d, s), np.float32)
        for c in range(bi * NBG, (bi + 1) * NBG):
            acc += res.results[c]["y"].astype(np.float32)
        y[bi] = acc.T + out_b[None, :]
    return y, res


def kernel(x, wqkv_w, wqkv_b, out_w, out_b):
    cfg = make_cfg(dt=os.environ.get("KMHA_DT", "bf16"))
    y, _ = run_mha(cfg, np.asarray(x, np.float32),
                   np.asarray(wqkv_w, np.float32),
                   np.asarray(wqkv_b, np.float32),
                   np.asarray(out_w, np.float32),
                   np.asarray(out_b, np.float32))
    return y


# revision 6
# speedup vs baseline: 1.2675x; 1.0405x over previous
"""MHA Trainium2 kernel v2: (batch x head-group) sharded across 8 NeuronCores.

Problem: B=2, S=2048, D=2560, H=32 heads, HD=80, partial rotary RD=32,
causal attention, fp32 reference; kernel computes in bf16 with f32 PSUM.

Core c handles batch c//4, heads (c%4)*8 .. +8.

Per-core phases:
  A1: q,k projection (x-tile stationary, weight moving) -> stage [tok, 1280],
      rope on DVE (3D strided APs batching all 8 heads), PE-transpose per head
      to qT/kT [80, s].
  A2: v projection -> stage_v [tok, 8*(80+1)] resident; per-head col 80 is a
      bias-only "ones" channel so PV yields the softmax denominator.
  C:  per (head, q-block): scoresT [k,q] on PE, exp on ACT (f32 PSUM -> bf16),
      causal mask multiply on diagonal tiles (DVE), PV accumulate [81, qb],
      reciprocal of row 80 + PE-broadcast + DVE multiply -> ctxT [80, s].
      ctxT repacked into [128, s] tiles via SBUF->SBUF DMA per head.
  D:  out-proj yT [d, s] partial (K=128 packed ctx), DMA out bf16.
  host: sum 4 partials per batch in f32, transpose, add out bias.
"""
import sys
import os

sys.path.insert(0, "/opt/trn_rl_repo")

import numpy as np
from contextlib import ExitStack

import concourse.bacc as bacc
import concourse.tile as tile
import concourse.mybir as mybir
from concourse.bass_utils import run_bass_kernel_spmd
from concourse.masks import make_identity

F32 = mybir.dt.float32
F32R = mybir.dt.float32r
BF16 = mybir.dt.bfloat16

B, S, D = 2, 2048, 2560
H, HD = 32, 80
RD = 32
ROPE_BASE = 10000.0
N_CORES = 8
NBG = 4  # cores per batch


def make_cfg(s=S, d=D, nh=H // NBG, hd=HD, rd=RD, qb=512, dt="bf16"):
    cfg = dict(s=s, d=d, nh=nh, hd=hd, rd=rd, qb=qb, dt=dt)
    cfg["n_st"] = s // 128
    cfg["n_kt"] = d // 128
    cfg["d_aug"] = cfg["n_kt"] * 128
    cfg["nqb"] = s // qb
    cfg["ndiag"] = qb // 128
    cfg["n_dt"] = d // 128
    cfg["jqk"] = 2 * nh * hd          # 1280
    cfg["vw"] = ((hd + 31) // 32) * 32 + 1  # 97: ones channel at 32-aligned col
    cfg["jv"] = nh * cfg["vw"]              # 776 (vA layout, incl. pads)
    cfg["jvc"] = nh * (hd + 1)              # 648 (compact weight cols)
    cfg["n_ct"] = (nh * hd) // 128    # 5 packed ctx tiles
    return cfg


def _dt(cfg):
    return {"bf16": BF16, "f32": F32, "f32r": F32R}[cfg["dt"]]


def _groups(total, step=512):
    out = []
    a = 0
    while a < total:
        out.append((a, min(a + step, total)))
        a += step
    return out


def build_program(cfg):
    s, d, nh, hd, rd = cfg["s"], cfg["d"], cfg["nh"], cfg["hd"], cfg["rd"]
    qb, n_st, n_kt = cfg["qb"], cfg["n_st"], cfg["n_kt"]
    nqb, ndiag, n_dt = cfg["nqb"], cfg["ndiag"], cfg["n_dt"]
    jqk, vw, jv, n_ct = cfg["jqk"], cfg["vw"], cfg["jv"], cfg["n_ct"]
    jvc = cfg["jvc"]
    DT = _dt(cfg)
    rh = rd // 2

    nc = bacc.Bacc(None, debug=False)

    xs_d = nc.declare_dram_parameter("xs", [n_st, 128, n_kt * 128], DT,
                                     isOutput=False)
    wqk_d = nc.declare_dram_parameter("wqk", [n_kt, 128, jqk], DT,
                                      isOutput=False)
    wv_d = nc.declare_dram_parameter("wv", [n_kt, 128, jvc], DT,
                                     isOutput=False)
    outw_d = nc.declare_dram_parameter("outw", [n_ct, 128, d], DT,
                                       isOutput=False)
    cos_d = nc.declare_dram_parameter("cosR", [128, n_st * nh * rh], DT,
                                      isOutput=False)
    sin_d = nc.declare_dram_parameter("sinR", [128, n_st * nh * rh], DT,
                                      isOutput=False)
    mask_d = nc.declare_dram_parameter("masks", [128, ndiag * qb], DT,
                                       isOutput=False)
    bqk_d = nc.declare_dram_parameter("bqk", [128, jqk], DT, isOutput=False)
    bv_d = nc.declare_dram_parameter("bv", [128, jvc], DT, isOutput=False)
    y_d = nc.declare_dram_parameter("y", [d, s], DT, isOutput=True)

    with tile.TileContext(nc) as tc, ExitStack() as top:
        top.enter_context(
            nc.allow_low_precision(reason="intentional bf16 storage"))
        glob = top.enter_context(tc.tile_pool(name="glob", bufs=1))
        identf = glob.tile([128, 128], F32)
        make_identity(nc, identf)
        if DT is F32:
            ident = identf
        else:
            ident = glob.tile([128, 128], DT)
            nc.vector.tensor_copy(ident, identf)
        ones1f = glob.tile([1, hd], F32)
        nc.vector.memset(ones1f, 1.0)
        ones1 = glob.tile([1, hd], F32R)
        nc.vector.tensor_copy(ones1, ones1f)
        cosR = glob.tile([128, n_st * nh * rh], DT)
        sinR = glob.tile([128, n_st * nh * rh], DT)
        masks = glob.tile([128, ndiag * qb], DT)
        bqk = glob.tile([128, jqk], DT)
        bv = glob.tile([128, jvc], DT)

        qt_pool = top.enter_context(tc.tile_pool(name="qt", bufs=1))
        qT = [qt_pool.tile([hd, s], DT, tag=f"q{h}", name=f"qT{h}")
              for h in range(nh)]
        kT = [qt_pool.tile([hd, s], DT, tag=f"k{h}", name=f"kT{h}")
              for h in range(nh)]
        vp = top.enter_context(tc.tile_pool(name="vp", bufs=1))
        vA = [vp.tile([128, jv], DT, tag=f"v{st}", name=f"vA{st}")
              for st in range(n_st)]
        for st in range(n_st):
            nc.vector.memset(vA[st], 0.0)

        # ---- phase A1: q,k projection + rope + transpose ----
        qk_groups = _groups(jqk)
        xsp = top.enter_context(tc.tile_pool(name="xsp", bufs=3))
        xs0 = xsp.tile([128, n_kt * 128], DT, tag="xs", name="xs_a1_0")
        nc.sync.dma_start(out=xs0, in_=xs_d[0])
        with ExitStack() as a1:
            wp = a1.enter_context(tc.tile_pool(name="w1", bufs=1))
            wqk = [wp.tile([128, jqk], DT, tag=f"w1_{kt}", name=f"wqk{kt}")
                   for kt in range(n_kt)]
            for kt in range(4):
                nc.sync.dma_start(out=wqk[kt], in_=wqk_d[kt])
            nc.sync.dma_start(out=cosR, in_=cos_d[:, :])
            nc.sync.dma_start(out=sinR, in_=sin_d[:, :])
            nc.sync.dma_start(out=masks, in_=mask_d[:, :])
            nc.sync.dma_start(out=bqk, in_=bqk_d[:, :])
            nc.sync.dma_start(out=bv, in_=bv_d[:, :])
            for kt in range(4, n_kt):
                nc.sync.dma_start(out=wqk[kt], in_=wqk_d[kt])
            stp = a1.enter_context(tc.tile_pool(name="stg1", bufs=2))
            psA = a1.enter_context(
                tc.tile_pool(name="psA", bufs=2, space="PSUM"))
            psT = a1.enter_context(
                tc.tile_pool(name="psT", bufs=2, space="PSUM"))
            rtp = a1.enter_context(tc.tile_pool(name="rt", bufs=2))
            for st in range(n_st):
                if st == 0:
                    xs = xs0
                else:
                    xs = xsp.tile([128, n_kt * 128], DT, tag="xs",
                                  name=f"xs_a1_{st}")
                    nc.sync.dma_start(out=xs, in_=xs_d[st])
                xs3 = xs.rearrange("p (t c) -> p t c", t=n_kt)
                ps = [psA.tile([128, g1 - g0], F32, tag=f"ps{gi}", name=f"psA{st}_{gi}")
                      for gi, (g0, g1) in enumerate(qk_groups)]
                for kt in range(n_kt):
                    for gi, (g0, g1) in enumerate(qk_groups):
                        nc.tensor.matmul(
                            ps[gi], xs3[:, kt, :], wqk[kt][:, g0:g1],
                            start=(kt == 0), stop=(kt == n_kt - 1))
                stage = stp.tile([128, jqk], DT, tag="stage")
                for gi, (g0, g1) in enumerate(qk_groups):
                    nc.vector.scalar_tensor_tensor(
                        out=stage[:, g0:g1], in0=ps[gi], scalar=1.0,
                        in1=bqk[:, g0:g1], op0=mybir.AluOpType.mult,
                        op1=mybir.AluOpType.add)
                # rope: all nh heads per op via strided 3D views
                cN = cosR[:, st * nh * rh:(st + 1) * nh * rh] \
                    .rearrange("p (h c) -> p h c", h=nh)
                sN = sinR[:, st * nh * rh:(st + 1) * nh * rh] \
                    .rearrange("p (h c) -> p h c", h=nh)
                for qk in range(2):
                    blk = stage[:, qk * nh * hd:(qk + 1) * nh * hd] \
                        .rearrange("p (h c) -> p h c", h=nh)
                    t1 = blk[:, :, 0:rh]
                    t2 = blk[:, :, rh:rd]
                    ta = rtp.tile([128, nh, rh], F32, tag="ta")
                    tb = rtp.tile([128, nh, rh], F32, tag="tb")
                    tg = rtp.tile([128, nh, rh], F32, tag="tg")
                    td = rtp.tile([128, nh, rh], F32, tag="td")
                    nc.vector.tensor_mul(ta, t1, cN)
                    nc.vector.tensor_mul(tb, t2, sN)
                    nc.vector.tensor_mul(tg, t1, sN)
                    nc.vector.tensor_mul(td, t2, cN)
                    nc.vector.tensor_sub(t1, ta, tb)
                    nc.vector.tensor_add(t2, tg, td)
                for h in range(nh):
                    for qk, dstT in ((0, qT), (1, kT)):
                        pt = psT.tile([hd, 128], DT, tag="pt")
                        nc.tensor.transpose(
                            pt, stage[:, qk * nh * hd + h * hd:
                                      qk * nh * hd + (h + 1) * hd], ident)
                        nc.vector.tensor_copy(
                            dstT[h][:, st * 128:(st + 1) * 128], pt)

        # ---- phase A2: v projection (+ ones channel per head) ----
        # weights are compact (81 cols/head); copies re-stride into the
        # padded vA layout with the ones channel landing at col 96.
        hw1 = hd + 1
        v_groups = [(0, 6 * hw1), (6 * hw1, nh * hw1)]  # head-aligned
        with ExitStack() as a2:
            wp2 = a2.enter_context(tc.tile_pool(name="w2", bufs=1))
            wv = [wp2.tile([128, jvc], DT, tag=f"w2_{kt}", name=f"wv{kt}")
                  for kt in range(n_kt)]
            for kt in range(n_kt):
                nc.sync.dma_start(out=wv[kt], in_=wv_d[kt])
            psA2 = a2.enter_context(
                tc.tile_pool(name="psA2", bufs=2, space="PSUM"))
            for st in range(n_st):
                xs = xsp.tile([128, n_kt * 128], DT, tag="xs",
                              name=f"xs_a2_{st}")
                nc.sync.dma_start(out=xs, in_=xs_d[st])
                xs3 = xs.rearrange("p (t c) -> p t c", t=n_kt)
                ps = [psA2.tile([128, g1 - g0], F32, tag=f"ps{gi}", name=f"psA2{st}_{gi}")
                      for gi, (g0, g1) in enumerate(v_groups)]
                for kt in range(n_kt):
                    for gi, (g0, g1) in enumerate(v_groups):
                        nc.tensor.matmul(
                            ps[gi], xs3[:, kt, :], wv[kt][:, g0:g1],
                            start=(kt == 0), stop=(kt == n_kt - 1))
                vA3 = vA[st].rearrange("p (h c) -> p h c", h=nh)
                for gi, (g0, g1) in enumerate(v_groups):
                    h0, h1 = g0 // hw1, g1 // hw1
                    ps3 = ps[gi].rearrange("p (h c) -> p h c", h=h1 - h0)
                    bv3 = bv[:, g0:g1].rearrange("p (h c) -> p h c",
                                                 h=h1 - h0)
                    nc.vector.scalar_tensor_tensor(
                        out=vA3[:, h0:h1, 0:hd], in0=ps3[:, :, 0:hd],
                        scalar=1.0, in1=bv3[:, :, 0:hd],
                        op0=mybir.AluOpType.mult, op1=mybir.AluOpType.add)
                    nc.vector.scalar_tensor_tensor(
                        out=vA3[:, h0:h1, vw - 1:vw],
                        in0=ps3[:, :, hd:hd + 1], scalar=1.0,
                        in1=bv3[:, :, hd:hd + 1],
                        op0=mybir.AluOpType.mult, op1=mybir.AluOpType.add)

        # ---- phase C: attention ----
        ctx_pool = top.enter_context(tc.tile_pool(name="ctx", bufs=1))
        ctxP = [ctx_pool.tile([128, s], DT, tag=f"cp{t}", name=f"ctxP{t}")
                for t in range(n_ct)]

        def repack(h, c0, c1, src_tile):
            g0 = h * hd
            r = g0
            while r < g0 + hd:
                ct = r // 128
                r1 = min((ct + 1) * 128, g0 + hd)
                nc.sync.dma_start(
                    out=ctxP[ct][r - ct * 128:r1 - ct * 128, c0:c1],
                    in_=src_tile[r - g0:r1 - g0, :])
                r = r1

        op = top.enter_context(tc.tile_pool(name="ow", bufs=1))
        ow = [op.tile([128, d], DT, tag=f"o{t}", name=f"ow{t}")
              for t in range(n_ct)]
        for t in range(n_ct):
            nc.sync.dma_start(out=ow[t], in_=outw_d[t])

        with ExitStack() as cstk:
            pp = cstk.enter_context(tc.tile_pool(name="pT", bufs=4))
            rp2 = cstk.enter_context(tc.tile_pool(name="rr", bufs=2))
            psS = cstk.enter_context(
                tc.tile_pool(name="psS", bufs=2, space="PSUM"))
            psC = cstk.enter_context(
                tc.tile_pool(name="psC", bufs=1, space="PSUM"))
            psD = cstk.enter_context(
                tc.tile_pool(name="psD", bufs=2, space="PSUM"))
            stp2 = cstk.enter_context(tc.tile_pool(name="st2", bufs=3))

            def finalize(h, q0, pctx):
                # copy den + ctx out of PSUM first: releases the pctx bank in
                # ~0.7us so the next block's accumulation can start while the
                # recip/broadcast/normalize chain runs from SBUF.
                den = rp2.tile([1, qb], F32, tag="rd")
                nc.vector.tensor_copy(den, pctx[vw - 1:vw, :])
                ctx_s = rp2.tile([hd, qb], DT, tag="cs")
                nc.vector.tensor_copy(ctx_s, pctx[0:hd, :])
                rden = rp2.tile([1, qb], F32, tag="rdf")
                nc.vector.reciprocal_approx_fast(out=rden, in_=den)
                rdenr = rp2.tile([1, qb], F32R, tag="rdr")
                nc.vector.tensor_copy(rdenr, rden)
                pbc = psD.tile([hd, qb], F32, tag="ps")
                nc.tensor.matmul(pbc, ones1, rdenr, start=True, stop=True)
                rb = rp2.tile([hd, qb], DT, tag="rb")
                nc.vector.tensor_copy(rb, pbc)
                cts = rp2.tile([hd, qb], DT, tag="ctso", name=f"cts{h}_{q0}")
                nc.vector.tensor_mul(cts, ctx_s, rb)
                repack(h, q0 * qb, (q0 + 1) * qb, cts)

            # D-phase work units are drip-fed into the C instruction
            # stream as PE filler: C alone is ACT(exp)-bound, which idles
            # the PE long enough for HAM to re-throttle the clock.
            pending_d = []

            def emit_d_unit(dt_i, sb):
                psy = psD.tile([128, qb], F32, tag="ps",
                               name=f"psy{dt_i}_{sb}")
                for ct in range(n_ct):
                    nc.tensor.matmul(
                        psy, ow[ct][:, dt_i * 128:(dt_i + 1) * 128],
                        ctxP[ct][:, sb * qb:(sb + 1) * qb],
                        start=(ct == 0), stop=(ct == n_ct - 1))
                ystage = stp2.tile([128, qb], DT, tag="y",
                                   name=f"yst{dt_i}_{sb}")
                nc.vector.tensor_copy(ystage, psy)
                nc.sync.dma_start(
                    out=y_d[dt_i * 128:(dt_i + 1) * 128,
                            sb * qb:(sb + 1) * qb],
                    in_=ystage)

            # per head: two sweeps of q0-pairs; kt-outer within a sweep.
            # pctx[parity] lives in 1 PSUM bank each; pss2 [128, 2*qb]
            # spans 2 banks and holds both q0s of the sweep per kt.
            dstep = 0
            for sw in range(nqb // 2):
                q0s = (2 * sw, 2 * sw + 1)
                for h in range(nh):
                    pctxs = {
                        q0: psC.tile([vw, qb], F32, tag=f"pc{q0 % 2}",
                                     name=f"pctx{h}_{q0}")
                        for q0 in q0s
                    }
                    nkt_max = (q0s[1] + 1) * qb // 128
                    for kt in range(nkt_max):
                        act = [q0 for q0 in q0s
                               if kt < (q0 + 1) * qb // 128]
                        pss = {}
                        for ci, q0 in enumerate(act):
                            pss[q0] = psS.tile([128, qb], F32,
                                               tag=f"ss{q0 % 2}",
                                               name=f"pss{h}_{kt}_{q0}")
                            nc.tensor.matmul(
                                pss[q0],
                                kT[h][:, kt * 128:(kt + 1) * 128],
                                qT[h][:, q0 * qb:(q0 + 1) * qb],
                                start=True, stop=True)
                        pTs = {}
                        for ci, q0 in enumerate(act):
                            pTs[q0] = pp.tile([128, qb], DT, tag="p",
                                              name=f"pT{h}_{kt}_{q0}")
                            nc.scalar.activation(
                                pTs[q0], pss[q0],
                                mybir.ActivationFunctionType.Exp)
                            od = kt * 128 - q0 * qb
                            if od >= 0:
                                oi = od // 128
                                nc.vector.tensor_mul(
                                    pTs[q0], pTs[q0],
                                    masks[:, oi * qb:(oi + 1) * qb])
                        for ci, q0 in enumerate(act):
                            nkt_q = (q0 + 1) * qb // 128
                            nc.tensor.matmul(
                                pctxs[q0], vA[kt][:, h * vw:(h + 1) * vw],
                                pTs[q0],
                                start=(kt == 0), stop=(kt == nkt_q - 1))
                            if kt == nkt_q - 1:
                                finalize(h, q0, pctxs[q0])
                        dstep += 1
                        if pending_d and dstep % 2 == 0:
                            emit_d_unit(*pending_d.pop(0))
                # this sweep's out-projection units become ready now
                pending_d.extend(
                    (dt_i, sb) for dt_i in range(n_dt) for sb in q0s)
            while pending_d:
                emit_d_unit(*pending_d.pop(0))

    nc.finalize()
    return nc


def prep_core_inputs(cfg, x, wqkv_w, wqkv_b, out_w, core):
    s, d, nh, hd, rd = cfg["s"], cfg["d"], cfg["nh"], cfg["hd"], cfg["rd"]
    qb, n_st, n_kt, d_aug = cfg["qb"], cfg["n_st"], cfg["n_kt"], cfg["d_aug"]
    ndiag, jqk, vw, jv = cfg["ndiag"], cfg["jqk"], cfg["vw"], cfg["jv"]
    n_ct = cfg["n_ct"]
    rh = rd // 2
    npdt = mybir.dt.np(_dt(cfg))

    bi = core // NBG
    hg = core % NBG
    heads = range(hg * nh, (hg + 1) * nh)
    rows = np.concatenate([np.arange(h * hd, (h + 1) * hd) for h in heads])
    scale = np.float32(1.0 / np.sqrt(hd))

    wq = wqkv_w[rows, :]
    bq = wqkv_b[rows]
    wk = wqkv_w[d + rows, :] * scale
    bk = wqkv_b[d + rows] * scale
    wv = wqkv_w[2 * d + rows, :]
    bv = wqkv_b[2 * d + rows]

    def wt_tiles(w):
        return np.ascontiguousarray(w.T).reshape(n_kt, 128, w.shape[0])

    wqk_arr = np.concatenate([wt_tiles(wq), wt_tiles(wk)], axis=2)
    bqk_arr = np.broadcast_to(
        np.concatenate([bq, bk])[None, :], (128, jqk))

    # v: compact 81 cols per head (80 weights + ones channel with zero
    # weights and bias 1); the device copies re-stride into the vA layout.
    jvc = cfg["jvc"]
    hw1 = hd + 1
    wva = np.zeros((d, jvc), np.float32)
    bva = np.zeros((jvc,), np.float32)
    for h in range(nh):
        wva[:, h * hw1:h * hw1 + hd] = wv[h * hd:(h + 1) * hd].T
        bva[h * hw1:h * hw1 + hd] = bv[h * hd:(h + 1) * hd]
        bva[h * hw1 + hd] = 1.0
    wv_arr = wva.reshape(n_kt, 128, jvc)
    bv_arr = np.broadcast_to(bva[None, :], (128, jvc))

    outw_arr = np.ascontiguousarray(
        out_w[:, rows].T.reshape(n_ct, 128, d))

    inv_freq = 1.0 / (ROPE_BASE ** (np.arange(0, rd, 2, dtype=np.float32) / rd))
    t = np.arange(s, dtype=np.float32)
    freqs = np.outer(t, inv_freq)  # [s, rh]
    # [128, n_st, nh, rh]: value depends on (token=st*128+p, freq i); repl. nh
    cos_arr = np.cos(freqs).astype(np.float32).reshape(n_st, 128, rh)
    cos_arr = np.broadcast_to(cos_arr[:, :, None, :], (n_st, 128, nh, rh))
    cos_arr = np.ascontiguousarray(
        cos_arr.transpose(1, 0, 2, 3).reshape(128, n_st * nh * rh))
    sin_arr = np.sin(freqs).astype(np.float32).reshape(n_st, 128, rh)
    sin_arr = np.broadcast_to(sin_arr[:, :, None, :], (n_st, 128, nh, rh))
    sin_arr = np.ascontiguousarray(
        sin_arr.transpose(1, 0, 2, 3).reshape(128, n_st * nh * rh))

    km = np.arange(128)[:, None]
    qm = np.arange(qb)[None, :]
    mask_arr = np.concatenate(
        [(qm >= i * 128 + km).astype(np.float32) for i in range(ndiag)],
        axis=1)

    xa = np.ascontiguousarray(x[bi].T)
    xs_arr = np.ascontiguousarray(
        xa.reshape(n_kt, 128, n_st, 128).transpose(2, 1, 0, 3)
    ).reshape(n_st, 128, n_kt * 128)

    return {
        "xs": xs_arr.astype(npdt),
        "wqk": np.ascontiguousarray(wqk_arr).astype(npdt),
        "wv": np.ascontiguousarray(wv_arr).astype(npdt),
        "outw": outw_arr.astype(npdt),
        "cosR": cos_arr.astype(npdt),
        "sinR": sin_arr.astype(npdt),
        "masks": np.ascontiguousarray(mask_arr).astype(npdt),
        "bqk": np.ascontiguousarray(bqk_arr).astype(npdt),
        "bv": np.ascontiguousarray(bv_arr).astype(npdt),
    }


_CACHE = {}


def run_mha(cfg, x, wqkv_w, wqkv_b, out_w, out_b, trace=False):
    key = tuple(sorted(cfg.items()))
    if key not in _CACHE:
        _CACHE[key] = build_program(cfg)
    nc = _CACHE[key]
    in_maps = [
        prep_core_inputs(cfg, x, wqkv_w, wqkv_b, out_w, c)
        for c in range(N_CORES)
    ]
    res = run_bass_kernel_spmd(nc, in_maps, core_ids=list(range(N_CORES)),
                               trace=trace)
    d, s = cfg["d"], cfg["s"]
    y = np.zeros((B, s, d), np.float32)
    for bi in range(B):
        acc = np.zeros((d, s), np.float32)
        for c in range(bi * NBG, (bi + 1) * NBG):
            acc += res.results[c]["y"].astype(np.float32)
        y[bi] = acc.T + out_b[None, :]
    return y, res


def kernel(x, wqkv_w, wqkv_b, out_w, out_b):
    cfg = make_cfg(dt=os.environ.get("KMHA_DT", "bf16"))
    y, _ = run_mha(cfg, np.asarray(x, np.float32),
                   np.asarray(wqkv_w, np.float32),
                   np.asarray(wqkv_b, np.float32),
                   np.asarray(out_w, np.float32),
                   np.asarray(out_b, np.float32))
    return y
